# revision 30
# baseline (speedup 1.0000x reference)
"""Trainium2 Bass kernel for nn_GatedAttention (linear attention with sigmoid
gate).

Strategy: shard the 16384 token rows across 8 cores (2048 each; cores 2b,2b+1
hold batch b). Per core, two phases:
  A: K,V projections (token-major) + per-head kv' = K^T [V|1] accumulated in
     persistent PSUM tiles over all local tokens (the ones column folds k_sum
     into kv'). kv matmuls run one m-tile behind the projections so the elu
     chain never stalls the PE.
  -- pairwise AllReduce of kv' between the two cores sharing a batch --
  B: Q,G projections (feature-major), out^T = kv'^T @ Q per head, normalizer
     z = SCALE/max(q.k_sum,eps) applied via tiny selector matmuls, gate, and
     the final output projection, all feature-major.
DMAs are issued in consumer order (X/wk/wv before phase A, wq/wg/wo/bg/sel
after the first m-tile, collective output after ch0's Q matmuls) so counting
semaphores never serialize the PE behind unrelated transfers.
Host transposes x to feature-major and pre-transposes weights; output returns
feature-major bf16 per-core slabs that the host transposes back.
"""
import sys

sys.path.insert(0, "/opt/trn_rl_repo")

import numpy as np
import ml_dtypes

B, N, DIM = 4, 4096, 1024
HEADS, DH = 16, 64
SCALE = DH ** -0.5
N_CORES = 8
TPC = B * N // N_CORES      # 2048 tokens per core
NMT = TPC // 128            # 16 m-tiles (phase A)
CHUNK = 512
NCH = TPC // CHUNK          # 4 chunks (phase B)
CLAMP = 1e-6 / SCALE

DT_MODE = "bf16"            # "bf16" | "f32r" | "f32"

_CACHE = {}


def _build(dt_mode=DT_MODE, reps=1):
    import concourse.bacc as bacc
    import concourse.bass as bass
    import concourse.tile as tile
    from concourse import mybir

    AF = mybir.ActivationFunctionType
    F32 = mybir.dt.float32
    DT = mybir.dt.bfloat16 if dt_mode == "bf16" else mybir.dt.float32

    def mm(ap):
        return ap.bitcast(mybir.dt.float32r) if dt_mode == "f32r" else ap

    ts = bass.ts

    nc = bacc.Bacc("TRN2", target_bir_lowering=False, debug=False,
                   num_devices=N_CORES)
    xt = nc.dram_tensor("xt", [DIM, TPC], DT, kind="ExternalInput")
    w_in = {}
    for nm in ("wk", "wv", "wq", "wg", "wo"):
        w_in[nm] = nc.dram_tensor(nm, [DIM, DIM], DT, kind="ExternalInput")
    bg_d = nc.dram_tensor("bg", [DIM], F32, kind="ExternalInput")
    y_d = nc.dram_tensor("y", [DIM, TPC], DT, kind="ExternalOutput")
    cc_in = nc.dram_tensor("cc_in", [128, 8, 65], F32)
    cc_out = nc.dram_tensor("cc_out", [128, 8, 65], F32)

    with tile.TileContext(nc, num_cores=N_CORES) as tc:
        with (
            tc.tile_pool(name="persist", bufs=1) as persist,
            tc.tile_pool(name="pb_big", bufs=2) as pb_big,
        ):
            X = persist.tile([128, 8, TPC], DT, tag="x")
            wsb = {}
            for nm in ("wq", "wg", "wo"):
                wsb[nm] = persist.tile([128, 8, DIM], DT, tag=nm, name=nm)
            bg_sb = persist.tile([128, 8], F32, tag="bg")
            sel_np = np.zeros((16, 8, 128), _np_dt(dt_mode))
            for p in range(8):
                sel_np[2 * p, p, 0:64] = 1.0
                sel_np[2 * p + 1, p, 64:128] = 1.0
            sel_d = nc.inline_tensor(sel_np, name="sel_const")
            sel = persist.tile([16, 8, 128], DT, tag="sel")

            for _rep in range(reps):
                _phases(nc, tc, bass, mybir, AF, F32, DT, mm, ts, X, wsb,
                        bg_sb, sel, sel_d, w_in, xt, bg_d, cc_in, cc_out, y_d,
                        tc_pools=(persist, pb_big))
    nc.compile()
    return nc


def _phases(nc, tc, bass, mybir, AF, F32, DT, mm, ts, X, wsb, bg_sb, sel,
            sel_d, w_in, xt, bg_d, cc_in, cc_out, y_d, tc_pools):
    persist, pb_big = tc_pools
    # ---------------- phase A ----------------
    with (
        tc.tile_pool(name="pa_w", bufs=1) as pa_w,
        tc.tile_pool(name="pa_tmp", bufs=2) as pa_tmp,
        tc.tile_pool(name="pa_ps", bufs=2, space="PSUM") as pa_ps,
        tc.tile_pool(name="kv_ps", bufs=1, space="PSUM") as kv_pool,
    ):
        for nm in ("wk", "wv"):
            wsb[nm] = pa_w.tile([128, 8, DIM], DT, tag=nm, name=nm)
        # consumer-ordered DMA issue: phase A inputs first, interleaved by
        # contraction chunk so the first m-tile can start ASAP (wv is only
        # needed once the first V-projection starts, ~3.4us after K).
        for i in range(8):
            nc.sync.dma_start(out=X[:, i, :], in_=xt.ap()[ts(i, 128), :])
            nc.sync.dma_start(out=wsb["wk"][:, i, :],
                              in_=w_in["wk"].ap()[ts(i, 128), :])
        for i in range(8):
            nc.sync.dma_start(out=wsb["wv"][:, i, :],
                              in_=w_in["wv"].ap()[ts(i, 128), :])

        # HAM warm-up: the PE clock sits at 1.2GHz until ~3.4us of sustained
        # matmul activity, and the input DMAs take ~12us to land AND pace
        # the first two m-tiles. Dummy matmuls on zeroed scratch fill the
        # initial window, and more are interleaved into the DMA-paced
        # m-tiles (below) so the activity window never sees an idle gap.
        warm_cm = tc.tile_pool(name="warm", bufs=1)
        warm_pool = warm_cm.__enter__()
        warm_ps_cm = tc.tile_pool(name="warm_ps", bufs=1, space="PSUM")
        warm_ps = warm_ps_cm.__enter__()
        wa = warm_pool.tile([128, 128], DT, tag="wa")
        wb = warm_pool.tile([128, 512], DT, tag="wb")
        nc.vector.memset(wa[:], 0.0)
        nc.vector.memset(wb[:], 0.0)
        wp = warm_ps.tile([128, 512], F32, tag="wp")
        for _ in range(14):
            nc.tensor.matmul(wp, mm(wa), mm(wb), start=True, stop=True)

        # persistent PSUM accumulators for kv': 2 tiles x [128, 4, 128]
        # (tile w holds heads 8w..8w+7: slice j rows 0:64 = head 8w+2j,
        #  rows 64:128 = head 8w+2j+1; j-slices padded to 128 floats so a
        #  matmul output never crosses a PSUM bank boundary)
        kv_ps = [kv_pool.tile([128, 4, 128], F32, tag=f"kv{w}",
                              name=f"kv{w}") for w in range(2)]
        ksb_hist = [None, None]
        vp_hist = [None, None]

        def kv_mms(mt):
            ksb_o = ksb_hist[mt % 2]
            vp_o = vp_hist[mt % 2]
            for w in range(2):
                for j in range(4):
                    for c in range(2):
                        h = 8 * w + 2 * j + c
                        # start only on the FIRST matmul touching this bank's
                        # partition plane: start_tensor_calc marks the whole
                        # 2KB zero-region pending, so a second start=True in
                        # the same bank would re-poison already-written
                        # slices and turn later accumulates into overwrites.
                        nc.tensor.matmul(
                            kv_ps[w][64 * c:64 * c + 64, j, 0:65],
                            mm(ksb_o[:, ts(h, 64)]),
                            mm(vp_o[:, h, :]),
                            start=(mt == 0 and j == 0),
                            stop=(mt == NMT - 1),
                            skip_group_check=True,
                        )

        for mt in range(NMT):
            msl = ts(mt, 128)
            kps = pa_ps.tile([128, 1024], F32, tag="proj")
            for i in range(8):
                for o in range(2):
                    nc.tensor.matmul(
                        kps[:, ts(o, 512)],
                        mm(X[:, i, msl]),
                        mm(wsb["wk"][:, i, ts(o, 512)]),
                        start=(i == 0), stop=(i == 7),
                    )
                if mt < 2:
                    nc.tensor.matmul(wp, mm(wa), mm(wb), start=True,
                                     stop=True)
            if mt == 1:
                # phase-B inputs: issued after the first m-tile's matmuls so
                # phase-A waits never count these transfers.
                bg_ap = bg_d.ap()
                nc.sync.dma_start(
                    out=bg_sb[:],
                    in_=bass.AP(tensor=bg_ap.tensor, offset=0,
                                ap=[[1, 128], [128, 8]]),
                )
                nc.sync.dma_start(out=sel[:], in_=sel_d.ap())
                for i in range(8):
                    for nm in ("wq", "wg", "wo"):
                        nc.sync.dma_start(out=wsb[nm][:, i, :],
                                          in_=w_in[nm].ap()[ts(i, 128), :])
            r1 = pa_tmp.tile([128, 1024], F32, tag="r1")
            nc.scalar.activation(r1, kps, AF.Relu)
            m1 = pa_tmp.tile([128, 1024], F32, tag="m1")
            nc.vector.tensor_scalar_min(m1, kps, 0.0)
            e1 = pa_tmp.tile([128, 1024], F32, tag="e1")
            nc.scalar.activation(e1, m1, AF.Exp)
            ksb = pa_tmp.tile([128, 1024], DT, tag="ksb")
            nc.gpsimd.tensor_add(ksb, r1, e1)
            ksb_hist[mt % 2] = ksb

            vps = pa_ps.tile([128, 16, 64], F32, tag="proj")
            for i in range(8):
                for o in range(2):
                    nc.tensor.matmul(
                        vps[:, ts(o, 8), :],
                        mm(X[:, i, msl]),
                        mm(wsb["wv"][:, i, ts(o, 512)]),
                        start=(i == 0), stop=(i == 7),
                    )
                if mt < 2:
                    nc.tensor.matmul(wp, mm(wa), mm(wb), start=True,
                                     stop=True)
            vp = pa_tmp.tile([128, 16, 65], DT, tag="vp")
            nc.vector.memset(vp[:, :, 64:65], 1.0)
            nc.scalar.copy(vp[:, :, 0:64], vps[:, :, :])
            vp_hist[mt % 2] = vp

            if mt > 0:
                kv_mms(mt - 1)
        kv_mms(NMT - 1)
        warm_ps_cm.__exit__(None, None, None)
        warm_cm.__exit__(None, None, None)

        kv_sb = pa_tmp.tile([128, 8, 65], F32, tag="kv_sb", bufs=1,
                            name="kv_sb")
        for w in range(2):
            nc.vector.tensor_copy(kv_sb[:, 4 * w:4 * w + 4, :],
                                  kv_ps[w][:, :, 0:65])
        nc.sync.dma_start(out=cc_in.ap()[:, :, :], in_=kv_sb[:])

    nc.gpsimd.collective_compute(
        "AllReduce",
        mybir.AluOpType.add,
        replica_groups=[[0, 1], [2, 3], [4, 5], [6, 7]],
        ins=[cc_in.ap().opt()],
        outs=[cc_out.ap().opt()],
    )

    # ---------------- phase B ----------------
    with (
        tc.tile_pool(name="pb_tmp", bufs=2) as pb_tmp,
        tc.tile_pool(name="pb_small", bufs=1) as pb_small,
        tc.tile_pool(name="pb_qg", bufs=1) as pb_qg,
    ):
        # collective results live in the persist pool: fresh SBUF, so the
        # kvf DMA has no write-after-read wait on phase-A consumers.
        kvf = persist.tile([128, 8, 65], F32, tag="kvf")
        kvb = persist.tile([128, 8, 65], DT, tag="kvb")
        ksd = persist.tile([128, 8, 16], DT, tag="ksd")

        ps_proj_cm = tc.tile_pool(name="ps_proj", bufs=2, space="PSUM")
        ps_proj = ps_proj_cm.__enter__()

        def proj_block(p, csl, qsb, gsb, which):
            pps = ps_proj.tile([128, CHUNK], F32, tag="proj")
            wname = "wq" if which == "q" else "wg"
            for i in range(8):
                nc.tensor.matmul(
                    pps, mm(wsb[wname][:, i, ts(p, 128)]),
                    mm(X[:, i, csl]),
                    start=(i == 0), stop=(i == 7),
                )
            if which == "q":
                r1 = pb_tmp.tile([128, CHUNK], F32, tag="br1")
                nc.scalar.activation(r1, pps, AF.Relu)
                m1 = pb_tmp.tile([128, CHUNK], F32, tag="bm1")
                nc.vector.tensor_scalar_min(m1, pps, 0.0)
                e1 = pb_tmp.tile([128, CHUNK], F32, tag="be1")
                nc.scalar.activation(e1, m1, AF.Exp)
                nc.gpsimd.tensor_add(qsb[:, p, :], r1, e1)
            else:
                nc.scalar.activation(gsb[:, p, :], pps, AF.Sigmoid,
                                     bias=bg_sb[:, p:p + 1])

        # ---- pass 1: Q and G projections for ALL chunks (collective-free
        # PE work that covers the AllReduce round-trip) ----
        qsbs, gsbs = [], []
        for ch in range(NCH):
            csl = ts(ch, CHUNK)
            qsb = pb_qg.tile([128, 8, CHUNK], DT, tag=f"qsb{ch}")
            gsb = pb_qg.tile([128, 8, CHUNK], DT, tag=f"gsb{ch}")
            qsbs.append(qsb)
            gsbs.append(gsb)
            for p in range(8):
                proj_block(p, csl, qsb, gsb, "q")
            if ch == 0:
                # collective output: DVE-only prep (no ACT/PE consumers
                # before the gate below).
                nc.sync.dma_start(out=kvf[:], in_=cc_out.ap()[:, :, :])
                nc.vector.tensor_copy(kvb, kvf)
                nc.vector.memset(ksd[:], 0.0)
                for p in range(8):
                    nc.vector.tensor_scalar_mul(
                        ksd[0:64, p, 2 * p:2 * p + 1],
                        kvf[0:64, p, 64:65], 1.0 / SCALE)
                    nc.vector.tensor_scalar_mul(
                        ksd[64:128, p, 2 * p + 1:2 * p + 2],
                        kvf[64:128, p, 64:65], 1.0 / SCALE)
            for p in range(8):
                proj_block(p, csl, qsb, gsb, "g")

        ps_proj_cm.__exit__(None, None, None)

        # ---- scheduling gate: ksd2/kvb2 = ksd/kvb + 0*gsb[last chunk].
        # The Tile scheduler orders PE instructions by ITS readiness model,
        # which does not know the AllReduce latency; without this gate it
        # hoists the first collective-dependent matmul right behind chunk
        # 0's projections and the in-order PE queue head-blocks on the
        # collective. The zero-valued dependency on the LAST chunk's gate
        # projection forces every collective-dependent matmul to be
        # scheduled after all projection work.
        # (small contiguous ops only: strided 3D tensor ops on Pool/DVE were
        # measured at 2-7.5us each and sat on the pass-1 -> pass-2 critical
        # path; the gate source is the LAST projection block's output slice)
        ksd2 = persist.tile([128, 8, 16], DT, tag="ksd2")
        kvb2 = persist.tile([128, 8, 65], DT, tag="kvb2")
        zl65 = pb_small.tile([128, 65], DT, tag="zl65")
        nc.gpsimd.tensor_scalar_mul(zl65, gsbs[NCH - 1][:, 7, 0:65], 0.0)
        for p in range(8):
            nc.vector.tensor_add(ksd2[:, p, :], ksd[:, p, :],
                                 zl65[:, 0:16])
            nc.vector.tensor_add(kvb2[:, p, :], kvb[:, p, :], zl65)

        ps_ops_cm = tc.tile_pool(name="ps_ops", bufs=2, space="PSUM")
        ps_z_cm = tc.tile_pool(name="ps_z", bufs=2, space="PSUM")
        ps_qk_cm = tc.tile_pool(name="ps_qk", bufs=2, space="PSUM")
        ps_y_cm = tc.tile_pool(name="ps_y", bufs=2, space="PSUM")
        ps_ops = ps_ops_cm.__enter__()
        ps_z = ps_z_cm.__enter__()
        ps_qk = ps_qk_cm.__enter__()
        ps_y = ps_y_cm.__enter__()

        # ---- pass 2: attention + output projection per chunk; each
        # chunk's qk runs one chunk ahead so the z reciprocal chain (DVE)
        # hides under the previous chunk's y matmuls ----
        def qk_mms(ch):
            qkps = ps_qk.tile([16, CHUNK], F32, tag="qk")
            for p in range(8):
                nc.tensor.matmul(
                    qkps, mm(ksd2[:, p, :]), mm(qsbs[ch][:, p, :]),
                    start=(p == 0), stop=(p == 7),
                    skip_group_check=True,
                )
            return qkps

        qkps_next = qk_mms(0)
        for ch in range(NCH):
            csl = ts(ch, CHUNK)
            qsb, gsb = qsbs[ch], gsbs[ch]
            qkps = qkps_next
            zq = pb_tmp.tile([16, CHUNK], F32, tag="zq")
            nc.vector.tensor_scalar_max(zq, qkps, CLAMP)
            zr = pb_tmp.tile([16, CHUNK], F32, tag="zr")
            nc.vector.reciprocal(zr, zq)
            zqr = pb_tmp.tile([16, CHUNK], DT, tag="zqr")
            nc.vector.tensor_copy(zqr, zr)

            asb = pb_big.tile([128, 8, CHUNK], DT, tag="asb")
            for p in range(8):
                ops_ = ps_ops.tile([128, CHUNK], F32, tag="ops")
                for rr in range(2):
                    pr = slice(64 * rr, 64 * rr + 64)
                    nc.tensor.matmul(
                        ops_[pr, :], mm(kvb2[pr, p, 0:64]),
                        mm(qsb[pr, p, :]),
                        start=True, stop=True,
                    )
                zbps = ps_z.tile([128, CHUNK], F32, tag="z")
                nc.tensor.matmul(zbps, mm(sel[:, p, :]), mm(zqr),
                                 start=True, stop=True)
                t1 = pb_tmp.tile([128, CHUNK], F32, tag="bt1")
                # each mul reads at most one PSUM operand (HW restriction)
                nc.vector.tensor_mul(t1, ops_, gsb[:, p, :])
                nc.vector.tensor_mul(asb[:, p, :], t1, zbps)
                if p == 1 and ch + 1 < NCH:
                    qkps_next = qk_mms(ch + 1)

            for d in range(8):
                yps = ps_y.tile([128, CHUNK], F32, tag="y")
                for fi in range(8):
                    nc.tensor.matmul(
                        yps, mm(wsb["wo"][:, fi, ts(d, 128)]),
                        mm(asb[:, fi, :]),
                        start=(fi == 0), stop=(fi == 7),
                    )
                ysb = pb_tmp.tile([128, CHUNK], DT, tag="ysb")
                nc.scalar.copy(ysb, yps)
                nc.sync.dma_start(out=y_d.ap()[ts(d, 128), csl],
                                  in_=ysb[:])

        ps_y_cm.__exit__(None, None, None)
        ps_qk_cm.__exit__(None, None, None)
        ps_z_cm.__exit__(None, None, None)
        ps_ops_cm.__exit__(None, None, None)


def _np_dt(dt_mode):
    return ml_dtypes.bfloat16 if dt_mode == "bf16" else np.float32


def prep_inputs(x, Wq, Wk, Wv, Wg, bg, Wo, dt_mode=DT_MODE):
    npdt = _np_dt(dt_mode)
    x_f = np.ascontiguousarray(np.asarray(x, np.float32).reshape(B * N, DIM))
    w_t = {}
    for nm, W in (("wq", Wq), ("wk", Wk), ("wv", Wv), ("wg", Wg)):
        w_t[nm] = np.ascontiguousarray(
            np.asarray(W, np.float32).T).astype(npdt)
    w_t["wo"] = np.ascontiguousarray(
        np.asarray(Wo, np.float32).T).astype(npdt)
    bg_f = np.ascontiguousarray(np.asarray(bg, np.float32))
    in_maps = []
    for c in range(N_CORES):
        xt_c = np.ascontiguousarray(
            x_f[c * TPC:(c + 1) * TPC].T).astype(npdt)
        m = {"xt": xt_c, "bg": bg_f}
        m.update(w_t)
        in_maps.append(m)
    return in_maps


def unshard_output(y_parts):
    out = np.empty((B * N, DIM), np.float32)
    for c in range(N_CORES):
        out[c * TPC:(c + 1) * TPC] = np.asarray(y_parts[c], np.float32).T
    return out.reshape(B, N, DIM)


def get_nc(dt_mode=DT_MODE):
    key = ("nc", dt_mode)
    if key not in _CACHE:
        _CACHE[key] = _build(dt_mode)
    return _CACHE[key]


def kernel(x, Wq, Wk, Wv, Wg, bg, Wo):
    from concourse.bass_utils import run_bass_kernel_spmd

    nc = get_nc()
    in_maps = prep_inputs(x, Wq, Wk, Wv, Wg, bg, Wo)
    res = run_bass_kernel_spmd(nc, in_maps, core_ids=list(range(N_CORES)))
    return unshard_output([res.results[c]["y"] for c in range(N_CORES)])


# revision 33
# speedup vs baseline: 1.0387x; 1.0387x over previous
"""Trainium2 Bass kernel for nn_GatedAttention (linear attention with sigmoid
gate).

Strategy: shard the 16384 token rows across 8 cores (2048 each; cores 2b,2b+1
hold batch b). Per core, two phases:
  A: K,V projections (token-major) + per-head kv' = K^T [V|1] accumulated in
     persistent PSUM tiles over all local tokens (the ones column folds k_sum
     into kv'). kv matmuls run one m-tile behind the projections so the elu
     chain never stalls the PE.
  -- pairwise AllReduce of kv' between the two cores sharing a batch --
  B: Q,G projections (feature-major), out^T = kv'^T @ Q per head, normalizer
     z = SCALE/max(q.k_sum,eps) applied via tiny selector matmuls, gate, and
     the final output projection, all feature-major.
DMAs are issued in consumer order (X/wk/wv before phase A, wq/wg/wo/bg/sel
after the first m-tile, collective output after ch0's Q matmuls) so counting
semaphores never serialize the PE behind unrelated transfers.
Host transposes x to feature-major and pre-transposes weights; output returns
feature-major bf16 per-core slabs that the host transposes back.
"""
import sys

sys.path.insert(0, "/opt/trn_rl_repo")

import numpy as np
import ml_dtypes

B, N, DIM = 4, 4096, 1024
HEADS, DH = 16, 64
SCALE = DH ** -0.5
N_CORES = 8
TPC = B * N // N_CORES      # 2048 tokens per core
NMT = TPC // 128            # 16 m-tiles (phase A)
CHUNK = 512
NCH = TPC // CHUNK          # 4 chunks (phase B)
CLAMP = 1e-6 / SCALE

DT_MODE = "bf16"            # "bf16" | "f32r" | "f32"

_CACHE = {}


def _build(dt_mode=DT_MODE, reps=1):
    import concourse.bacc as bacc
    import concourse.bass as bass
    import concourse.tile as tile
    from concourse import mybir

    AF = mybir.ActivationFunctionType
    F32 = mybir.dt.float32
    DT = mybir.dt.bfloat16 if dt_mode == "bf16" else mybir.dt.float32

    def mm(ap):
        return ap.bitcast(mybir.dt.float32r) if dt_mode == "f32r" else ap

    ts = bass.ts

    nc = bacc.Bacc("TRN2", target_bir_lowering=False, debug=False,
                   num_devices=N_CORES)
    xt = nc.dram_tensor("xt", [DIM, TPC], DT, kind="ExternalInput")
    w_in = {}
    for nm in ("wk", "wv", "wq", "wg", "wo"):
        w_in[nm] = nc.dram_tensor(nm, [DIM, DIM], DT, kind="ExternalInput")
    bg_d = nc.dram_tensor("bg", [DIM], F32, kind="ExternalInput")
    y_d = nc.dram_tensor("y", [DIM, TPC], DT, kind="ExternalOutput")
    cc_in = nc.dram_tensor("cc_in", [128, 8, 65], F32)
    cc_out = nc.dram_tensor("cc_out", [128, 8, 65], F32)

    with tile.TileContext(nc, num_cores=N_CORES) as tc:
        with (
            tc.tile_pool(name="persist", bufs=1) as persist,
            tc.tile_pool(name="pb_big", bufs=2) as pb_big,
        ):
            X = persist.tile([128, 8, TPC], DT, tag="x")
            wsb = {}
            for nm in ("wq", "wg", "wo"):
                wsb[nm] = persist.tile([128, 8, DIM], DT, tag=nm, name=nm)
            bg_sb = persist.tile([128, 8], F32, tag="bg")
            sel_np = np.zeros((16, 8, 128), _np_dt(dt_mode))
            for p in range(8):
                sel_np[2 * p, p, 0:64] = 1.0
                sel_np[2 * p + 1, p, 64:128] = 1.0
            sel_d = nc.inline_tensor(sel_np, name="sel_const")
            sel = persist.tile([16, 8, 128], DT, tag="sel")

            for _rep in range(reps):
                _phases(nc, tc, bass, mybir, AF, F32, DT, mm, ts, X, wsb,
                        bg_sb, sel, sel_d, w_in, xt, bg_d, cc_in, cc_out, y_d,
                        tc_pools=(persist, pb_big))
    nc.compile()
    return nc


def _phases(nc, tc, bass, mybir, AF, F32, DT, mm, ts, X, wsb, bg_sb, sel,
            sel_d, w_in, xt, bg_d, cc_in, cc_out, y_d, tc_pools):
    persist, pb_big = tc_pools
    # ---------------- phase A ----------------
    with (
        tc.tile_pool(name="pa_w", bufs=1) as pa_w,
        tc.tile_pool(name="pa_tmp", bufs=2) as pa_tmp,
        tc.tile_pool(name="pa_ps", bufs=2, space="PSUM") as pa_ps,
        tc.tile_pool(name="kv_ps", bufs=1, space="PSUM") as kv_pool,
    ):
        for nm in ("wk", "wv"):
            wsb[nm] = pa_w.tile([128, 8, DIM], DT, tag=nm, name=nm)
        # consumer-ordered DMA issue: phase A inputs first, interleaved by
        # contraction chunk so the first m-tile can start ASAP (wv is only
        # needed once the first V-projection starts, ~3.4us after K).
        for i in range(8):
            nc.sync.dma_start(out=X[:, i, :], in_=xt.ap()[ts(i, 128), :])
            nc.sync.dma_start(out=wsb["wk"][:, i, :],
                              in_=w_in["wk"].ap()[ts(i, 128), :])
        for i in range(8):
            nc.sync.dma_start(out=wsb["wv"][:, i, :],
                              in_=w_in["wv"].ap()[ts(i, 128), :])

        # HAM warm-up: the PE clock sits at 1.2GHz until ~3.4us of sustained
        # matmul activity, and the input DMAs take ~12us to land AND pace
        # the first two m-tiles. Dummy matmuls on zeroed scratch fill the
        # initial window, and more are interleaved into the DMA-paced
        # m-tiles (below) so the activity window never sees an idle gap.
        warm_cm = tc.tile_pool(name="warm", bufs=1)
        warm_pool = warm_cm.__enter__()
        warm_ps_cm = tc.tile_pool(name="warm_ps", bufs=1, space="PSUM")
        warm_ps = warm_ps_cm.__enter__()
        wa = warm_pool.tile([128, 128], DT, tag="wa")
        wb = warm_pool.tile([128, 512], DT, tag="wb")
        nc.vector.memset(wa[:], 0.0)
        nc.vector.memset(wb[:], 0.0)
        wp = warm_ps.tile([128, 512], F32, tag="wp")
        for _ in range(14):
            nc.tensor.matmul(wp, mm(wa), mm(wb), start=True, stop=True)

        # persistent PSUM accumulators for kv': 2 tiles x [128, 4, 128]
        # (tile w holds heads 8w..8w+7: slice j rows 0:64 = head 8w+2j,
        #  rows 64:128 = head 8w+2j+1; j-slices padded to 128 floats so a
        #  matmul output never crosses a PSUM bank boundary)
        kv_ps = [kv_pool.tile([128, 4, 128], F32, tag=f"kv{w}",
                              name=f"kv{w}") for w in range(2)]
        ksb_hist = [None, None]
        vp_hist = [None, None]

        def kv_mms(mt):
            ksb_o = ksb_hist[mt % 2]
            vp_o = vp_hist[mt % 2]
            for w in range(2):
                for j in range(4):
                    for c in range(2):
                        h = 8 * w + 2 * j + c
                        # start only on the FIRST matmul touching this bank's
                        # partition plane: start_tensor_calc marks the whole
                        # 2KB zero-region pending, so a second start=True in
                        # the same bank would re-poison already-written
                        # slices and turn later accumulates into overwrites.
                        nc.tensor.matmul(
                            kv_ps[w][64 * c:64 * c + 64, j, 0:65],
                            mm(ksb_o[:, ts(h, 64)]),
                            mm(vp_o[:, h, :]),
                            start=(mt == 0 and j == 0),
                            stop=(mt == NMT - 1),
                            skip_group_check=True,
                        )

        for mt in range(NMT):
            msl = ts(mt, 128)
            kps = pa_ps.tile([128, 1024], F32, tag="proj")
            for i in range(8):
                for o in range(2):
                    nc.tensor.matmul(
                        kps[:, ts(o, 512)],
                        mm(X[:, i, msl]),
                        mm(wsb["wk"][:, i, ts(o, 512)]),
                        start=(i == 0), stop=(i == 7),
                    )
                if mt < 2:
                    nc.tensor.matmul(wp, mm(wa), mm(wb), start=True,
                                     stop=True)
            if mt == 1:
                # phase-B inputs: issued after the first m-tile's matmuls so
                # phase-A waits never count these transfers.
                bg_ap = bg_d.ap()
                nc.sync.dma_start(
                    out=bg_sb[:],
                    in_=bass.AP(tensor=bg_ap.tensor, offset=0,
                                ap=[[1, 128], [128, 8]]),
                )
                nc.sync.dma_start(out=sel[:], in_=sel_d.ap())
                for i in range(8):
                    for nm in ("wq", "wg", "wo"):
                        nc.sync.dma_start(out=wsb[nm][:, i, :],
                                          in_=w_in[nm].ap()[ts(i, 128), :])
            r1 = pa_tmp.tile([128, 1024], F32, tag="r1")
            nc.scalar.activation(r1, kps, AF.Relu)
            m1 = pa_tmp.tile([128, 1024], F32, tag="m1")
            nc.vector.tensor_scalar_min(m1, kps, 0.0)
            e1 = pa_tmp.tile([128, 1024], F32, tag="e1")
            nc.scalar.activation(e1, m1, AF.Exp)
            ksb = pa_tmp.tile([128, 1024], DT, tag="ksb")
            nc.gpsimd.tensor_add(ksb, r1, e1)
            ksb_hist[mt % 2] = ksb

            vps = pa_ps.tile([128, 16, 64], F32, tag="proj")
            for i in range(8):
                for o in range(2):
                    nc.tensor.matmul(
                        vps[:, ts(o, 8), :],
                        mm(X[:, i, msl]),
                        mm(wsb["wv"][:, i, ts(o, 512)]),
                        start=(i == 0), stop=(i == 7),
                    )
                if mt < 2:
                    nc.tensor.matmul(wp, mm(wa), mm(wb), start=True,
                                     stop=True)
            vp = pa_tmp.tile([128, 16, 65], DT, tag="vp")
            nc.vector.memset(vp[:, :, 64:65], 1.0)
            nc.scalar.copy(vp[:, :, 0:64], vps[:, :, :])
            vp_hist[mt % 2] = vp

            if mt > 0:
                kv_mms(mt - 1)
        kv_mms(NMT - 1)
        warm_ps_cm.__exit__(None, None, None)
        warm_cm.__exit__(None, None, None)

        kv_sb = pa_tmp.tile([128, 8, 65], F32, tag="kv_sb", bufs=1,
                            name="kv_sb")
        for w in range(2):
            nc.vector.tensor_copy(kv_sb[:, 4 * w:4 * w + 4, :],
                                  kv_ps[w][:, :, 0:65])
        nc.sync.dma_start(out=cc_in.ap()[:, :, :], in_=kv_sb[:])

    nc.gpsimd.collective_compute(
        "AllReduce",
        mybir.AluOpType.add,
        replica_groups=[[0, 1], [2, 3], [4, 5], [6, 7]],
        ins=[cc_in.ap().opt()],
        outs=[cc_out.ap().opt()],
    )

    # ---------------- phase B ----------------
    with (
        tc.tile_pool(name="pb_tmp", bufs=2) as pb_tmp,
        tc.tile_pool(name="pb_small", bufs=1) as pb_small,
        tc.tile_pool(name="pb_qg", bufs=1) as pb_qg,
    ):
        # collective results live in the persist pool: fresh SBUF, so the
        # kvf DMA has no write-after-read wait on phase-A consumers.
        kvf = persist.tile([128, 8, 65], F32, tag="kvf")
        kvb = persist.tile([128, 8, 65], DT, tag="kvb")
        ksd = persist.tile([128, 8, 16], DT, tag="ksd")

        ps_proj_cm = tc.tile_pool(name="ps_proj", bufs=3, space="PSUM")
        ps_proj = ps_proj_cm.__enter__()

        def proj_block(p, csl, qsb, gsb, which):
            pps = ps_proj.tile([128, CHUNK], F32, tag="proj")
            wname = "wq" if which == "q" else "wg"
            for i in range(8):
                nc.tensor.matmul(
                    pps, mm(wsb[wname][:, i, ts(p, 128)]),
                    mm(X[:, i, csl]),
                    start=(i == 0), stop=(i == 7),
                )
            if which == "q":
                r1 = pb_tmp.tile([128, CHUNK], F32, tag="br1")
                nc.scalar.activation(r1, pps, AF.Relu)
                m1 = pb_tmp.tile([128, CHUNK], F32, tag="bm1")
                nc.vector.tensor_scalar_min(m1, pps, 0.0)
                e1 = pb_tmp.tile([128, CHUNK], F32, tag="be1")
                nc.scalar.activation(e1, m1, AF.Exp)
                nc.gpsimd.tensor_add(qsb[:, p, :], r1, e1)
            else:
                nc.scalar.activation(gsb[:, p, :], pps, AF.Sigmoid,
                                     bias=bg_sb[:, p:p + 1])

        # ---- pass 1: Q and G projections for ALL chunks (collective-free
        # PE work that covers the AllReduce round-trip) ----
        qsbs, gsbs = [], []
        for ch in range(NCH):
            csl = ts(ch, CHUNK)
            qsb = pb_qg.tile([128, 8, CHUNK], DT, tag=f"qsb{ch}")
            gsb = pb_qg.tile([128, 8, CHUNK], DT, tag=f"gsb{ch}")
            qsbs.append(qsb)
            gsbs.append(gsb)
            for p in range(8):
                proj_block(p, csl, qsb, gsb, "q")
            if ch == 0:
                nc.sync.dma_start(out=kvf[:], in_=cc_out.ap()[:, :, :])
            for p in range(8):
                proj_block(p, csl, qsb, gsb, "g")

        ps_proj_cm.__exit__(None, None, None)

        # ---- collective prep + scheduling gate, ALL on the Pool engine.
        # No pass-1 matmul ever waits on Pool completions, so these
        # collective-dependent ops cannot poison pass-1 counting-semaphore
        # thresholds (they did when placed on ACT or DVE). ksd2/kvb2 gain a
        # zero-valued dependency on the LAST projection block's output:
        # the Tile scheduler's readiness model ignores AllReduce latency
        # and would otherwise hoist the first collective-dependent matmul
        # right behind chunk 0's projections, head-blocking the in-order
        # PE queue on the collective.
        nc.gpsimd.tensor_copy(kvb, kvf)
        nc.gpsimd.memset(ksd[:], 0.0)
        for p in range(8):
            nc.gpsimd.tensor_scalar_mul(
                ksd[0:64, p, 2 * p:2 * p + 1],
                kvf[0:64, p, 64:65], 1.0 / SCALE)
            nc.gpsimd.tensor_scalar_mul(
                ksd[64:128, p, 2 * p + 1:2 * p + 2],
                kvf[64:128, p, 64:65], 1.0 / SCALE)
        ksd2 = persist.tile([128, 8, 16], DT, tag="ksd2")
        kvb2 = persist.tile([128, 8, 65], DT, tag="kvb2")
        zl65 = pb_small.tile([128, 65], DT, tag="zl65")
        nc.gpsimd.tensor_scalar_mul(zl65, gsbs[NCH - 1][:, 7, 0:65], 0.0)
        for p in range(8):
            nc.gpsimd.tensor_add(ksd2[:, p, :], ksd[:, p, :],
                                 zl65[:, 0:16])
            nc.gpsimd.tensor_add(kvb2[:, p, :], kvb[:, p, :], zl65)

        ps_ops_cm = tc.tile_pool(name="ps_ops", bufs=2, space="PSUM")
        ps_z_cm = tc.tile_pool(name="ps_z", bufs=2, space="PSUM")
        ps_qk_cm = tc.tile_pool(name="ps_qk", bufs=2, space="PSUM")
        ps_y_cm = tc.tile_pool(name="ps_y", bufs=2, space="PSUM")
        ps_ops = ps_ops_cm.__enter__()
        ps_z = ps_z_cm.__enter__()
        ps_qk = ps_qk_cm.__enter__()
        ps_y = ps_y_cm.__enter__()

        # ---- pass 2: attention + output projection per chunk; each
        # chunk's qk runs one chunk ahead so the z reciprocal chain (DVE)
        # hides under the previous chunk's y matmuls ----
        def qk_mms(ch):
            qkps = ps_qk.tile([16, CHUNK], F32, tag="qk")
            for p in range(8):
                nc.tensor.matmul(
                    qkps, mm(ksd2[:, p, :]), mm(qsbs[ch][:, p, :]),
                    start=(p == 0), stop=(p == 7),
                    skip_group_check=True,
                )
            return qkps

        qkps_next = qk_mms(0)
        for ch in range(NCH):
            csl = ts(ch, CHUNK)
            qsb, gsb = qsbs[ch], gsbs[ch]
            qkps = qkps_next
            zq = pb_tmp.tile([16, CHUNK], F32, tag="zq")
            nc.vector.tensor_scalar_max(zq, qkps, CLAMP)
            zr = pb_tmp.tile([16, CHUNK], F32, tag="zr")
            nc.vector.reciprocal(zr, zq)
            zqr = pb_tmp.tile([16, CHUNK], DT, tag="zqr")
            nc.vector.tensor_copy(zqr, zr)

            asb = pb_big.tile([128, 8, CHUNK], DT, tag="asb")
            for p in range(8):
                ops_ = ps_ops.tile([128, CHUNK], F32, tag="ops")
                for rr in range(2):
                    pr = slice(64 * rr, 64 * rr + 64)
                    nc.tensor.matmul(
                        ops_[pr, :], mm(kvb2[pr, p, 0:64]),
                        mm(qsb[pr, p, :]),
                        start=True, stop=True,
                    )
                zbps = ps_z.tile([128, CHUNK], F32, tag="z")
                nc.tensor.matmul(zbps, mm(sel[:, p, :]), mm(zqr),
                                 start=True, stop=True)
                t1 = pb_tmp.tile([128, CHUNK], F32, tag="bt1")
                # each mul reads at most one PSUM operand (HW restriction)
                nc.vector.tensor_mul(t1, ops_, gsb[:, p, :])
                nc.vector.tensor_mul(asb[:, p, :], t1, zbps)
                if p == 1 and ch + 1 < NCH:
                    qkps_next = qk_mms(ch + 1)

            for d in range(8):
                yps = ps_y.tile([128, CHUNK], F32, tag="y")
                for fi in range(8):
                    nc.tensor.matmul(
                        yps, mm(wsb["wo"][:, fi, ts(d, 128)]),
                        mm(asb[:, fi, :]),
                        start=(fi == 0), stop=(fi == 7),
                    )
                ysb = pb_tmp.tile([128, CHUNK], DT, tag="ysb")
                nc.scalar.copy(ysb, yps)
                nc.sync.dma_start(out=y_d.ap()[ts(d, 128), csl],
                                  in_=ysb[:])

        ps_y_cm.__exit__(None, None, None)
        ps_qk_cm.__exit__(None, None, None)
        ps_z_cm.__exit__(None, None, None)
        ps_ops_cm.__exit__(None, None, None)


def _np_dt(dt_mode):
    return ml_dtypes.bfloat16 if dt_mode == "bf16" else np.float32


def prep_inputs(x, Wq, Wk, Wv, Wg, bg, Wo, dt_mode=DT_MODE):
    npdt = _np_dt(dt_mode)
    x_f = np.ascontiguousarray(np.asarray(x, np.float32).reshape(B * N, DIM))
    w_t = {}
    for nm, W in (("wq", Wq), ("wk", Wk), ("wv", Wv), ("wg", Wg)):
        w_t[nm] = np.ascontiguousarray(
            np.asarray(W, np.float32).T).astype(npdt)
    w_t["wo"] = np.ascontiguousarray(
        np.asarray(Wo, np.float32).T).astype(npdt)
    bg_f = np.ascontiguousarray(np.asarray(bg, np.float32))
    in_maps = []
    for c in range(N_CORES):
        xt_c = np.ascontiguousarray(
            x_f[c * TPC:(c + 1) * TPC].T).astype(npdt)
        m = {"xt": xt_c, "bg": bg_f}
        m.update(w_t)
        in_maps.append(m)
    return in_maps


def unshard_output(y_parts):
    out = np.empty((B * N, DIM), np.float32)
    for c in range(N_CORES):
        out[c * TPC:(c + 1) * TPC] = np.asarray(y_parts[c], np.float32).T
    return out.reshape(B, N, DIM)


def get_nc(dt_mode=DT_MODE):
    key = ("nc", dt_mode)
    if key not in _CACHE:
        _CACHE[key] = _build(dt_mode)
    return _CACHE[key]


def kernel(x, Wq, Wk, Wv, Wg, bg, Wo):
    from concourse.bass_utils import run_bass_kernel_spmd

    nc = get_nc()
    in_maps = prep_inputs(x, Wq, Wk, Wv, Wg, bg, Wo)
    res = run_bass_kernel_spmd(nc, in_maps, core_ids=list(range(N_CORES)))
    return unshard_output([res.results[c]["y"] for c in range(N_CORES)])


# revision 36
# speedup vs baseline: 1.0625x; 1.0228x over previous
"""Trainium2 Bass kernel for nn_GatedAttention (linear attention with sigmoid
gate).

Strategy: shard the 16384 token rows across 8 cores (2048 each; cores 2b,2b+1
hold batch b). Per core, two phases:
  A: K,V projections (token-major) + per-head kv' = K^T [V|1] accumulated in
     persistent PSUM tiles over all local tokens (the ones column folds k_sum
     into kv'). kv matmuls run one m-tile behind the projections so the elu
     chain never stalls the PE.
  -- pairwise AllReduce of kv' between the two cores sharing a batch --
  B: Q,G projections (feature-major), out^T = kv'^T @ Q per head, normalizer
     z = SCALE/max(q.k_sum,eps) applied via tiny selector matmuls, gate, and
     the final output projection, all feature-major.
DMAs are issued in consumer order (X/wk/wv before phase A, wq/wg/wo/bg/sel
after the first m-tile, collective output after ch0's Q matmuls) so counting
semaphores never serialize the PE behind unrelated transfers.
Host transposes x to feature-major and pre-transposes weights; output returns
feature-major bf16 per-core slabs that the host transposes back.
"""
import sys

sys.path.insert(0, "/opt/trn_rl_repo")

import numpy as np
import ml_dtypes

B, N, DIM = 4, 4096, 1024
HEADS, DH = 16, 64
SCALE = DH ** -0.5
N_CORES = 8
TPC = B * N // N_CORES      # 2048 tokens per core
NMT = TPC // 128            # 16 m-tiles (phase A)
CHUNK = 512
NCH = TPC // CHUNK          # 4 chunks (phase B)
CLAMP = 1e-6 / SCALE

DT_MODE = "bf16"            # "bf16" | "f32r" | "f32"

_CACHE = {}


def _build(dt_mode=DT_MODE, reps=1):
    import concourse.bacc as bacc
    import concourse.bass as bass
    import concourse.tile as tile
    from concourse import mybir

    AF = mybir.ActivationFunctionType
    F32 = mybir.dt.float32
    DT = mybir.dt.bfloat16 if dt_mode == "bf16" else mybir.dt.float32

    def mm(ap):
        return ap.bitcast(mybir.dt.float32r) if dt_mode == "f32r" else ap

    ts = bass.ts

    nc = bacc.Bacc("TRN2", target_bir_lowering=False, debug=False,
                   num_devices=N_CORES)
    xt = nc.dram_tensor("xt", [DIM, TPC], DT, kind="ExternalInput")
    w_in = {}
    for nm in ("wk", "wv", "wq", "wg", "wo"):
        w_in[nm] = nc.dram_tensor(nm, [DIM, DIM], DT, kind="ExternalInput")
    bg_d = nc.dram_tensor("bg", [DIM], F32, kind="ExternalInput")
    y_d = nc.dram_tensor("y", [DIM, TPC], DT, kind="ExternalOutput")
    cc_in = nc.dram_tensor("cc_in", [128, 8, 65], F32)
    cc_out = nc.dram_tensor("cc_out", [128, 8, 65], F32)

    with tile.TileContext(nc, num_cores=N_CORES) as tc:
        with (
            tc.tile_pool(name="persist", bufs=1) as persist,
            tc.tile_pool(name="pb_big", bufs=2) as pb_big,
        ):
            X = persist.tile([128, 8, TPC], DT, tag="x")
            wsb = {}
            for nm in ("wq", "wg", "wo"):
                wsb[nm] = persist.tile([128, 8, DIM], DT, tag=nm, name=nm)
            bg_sb = persist.tile([128, 8], F32, tag="bg")
            sel_np = np.zeros((16, 8, 128), _np_dt(dt_mode))
            for p in range(8):
                sel_np[2 * p, p, 0:64] = 1.0
                sel_np[2 * p + 1, p, 64:128] = 1.0
            sel_d = nc.inline_tensor(sel_np, name="sel_const")
            sel = persist.tile([16, 8, 128], DT, tag="sel")

            for _rep in range(reps):
                _phases(nc, tc, bass, mybir, AF, F32, DT, mm, ts, X, wsb,
                        bg_sb, sel, sel_d, w_in, xt, bg_d, cc_in, cc_out, y_d,
                        tc_pools=(persist, pb_big))
    nc.compile()
    return nc


def _phases(nc, tc, bass, mybir, AF, F32, DT, mm, ts, X, wsb, bg_sb, sel,
            sel_d, w_in, xt, bg_d, cc_in, cc_out, y_d, tc_pools):
    persist, pb_big = tc_pools
    # ---------------- phase A ----------------
    with (
        tc.tile_pool(name="pa_w", bufs=1) as pa_w,
        tc.tile_pool(name="pa_tmp", bufs=2) as pa_tmp,
        tc.tile_pool(name="pa_ps", bufs=2, space="PSUM") as pa_ps,
        tc.tile_pool(name="kv_ps", bufs=1, space="PSUM") as kv_pool,
    ):
        for nm in ("wk", "wv"):
            wsb[nm] = pa_w.tile([128, 8, DIM], DT, tag=nm, name=nm)
        # consumer-ordered DMA issue: phase A inputs first, interleaved by
        # contraction chunk so the first m-tile can start ASAP (wv is only
        # needed once the first V-projection starts, ~3.4us after K).
        for i in range(8):
            nc.sync.dma_start(out=X[:, i, :], in_=xt.ap()[ts(i, 128), :])
            nc.sync.dma_start(out=wsb["wk"][:, i, :],
                              in_=w_in["wk"].ap()[ts(i, 128), :])
        for i in range(8):
            nc.sync.dma_start(out=wsb["wv"][:, i, :],
                              in_=w_in["wv"].ap()[ts(i, 128), :])

        # HAM warm-up: the PE clock sits at 1.2GHz until ~3.4us of sustained
        # matmul activity, and the input DMAs take ~12us to land AND pace
        # the first two m-tiles. Dummy matmuls on zeroed scratch fill the
        # initial window, and more are interleaved into the DMA-paced
        # m-tiles (below) so the activity window never sees an idle gap.
        warm_cm = tc.tile_pool(name="warm", bufs=1)
        warm_pool = warm_cm.__enter__()
        warm_ps_cm = tc.tile_pool(name="warm_ps", bufs=1, space="PSUM")
        warm_ps = warm_ps_cm.__enter__()
        wa = warm_pool.tile([128, 128], DT, tag="wa")
        wb = warm_pool.tile([128, 512], DT, tag="wb")
        nc.vector.memset(wa[:], 0.0)
        nc.vector.memset(wb[:], 0.0)
        wp = warm_ps.tile([128, 512], F32, tag="wp")
        for _ in range(14):
            nc.tensor.matmul(wp, mm(wa), mm(wb), start=True, stop=True)

        # persistent PSUM accumulators for kv': 2 tiles x [128, 4, 128]
        # (tile w holds heads 8w..8w+7: slice j rows 0:64 = head 8w+2j,
        #  rows 64:128 = head 8w+2j+1; j-slices padded to 128 floats so a
        #  matmul output never crosses a PSUM bank boundary)
        kv_ps = [kv_pool.tile([128, 4, 128], F32, tag=f"kv{w}",
                              name=f"kv{w}") for w in range(2)]
        ksb_hist = [None, None]
        vp_hist = [None, None]

        def kv_mms(mt):
            ksb_o = ksb_hist[mt % 2]
            vp_o = vp_hist[mt % 2]
            for w in range(2):
                for j in range(4):
                    for c in range(2):
                        h = 8 * w + 2 * j + c
                        # start only on the FIRST matmul touching this bank's
                        # partition plane: start_tensor_calc marks the whole
                        # 2KB zero-region pending, so a second start=True in
                        # the same bank would re-poison already-written
                        # slices and turn later accumulates into overwrites.
                        nc.tensor.matmul(
                            kv_ps[w][64 * c:64 * c + 64, j, 0:65],
                            mm(ksb_o[:, ts(h, 64)]),
                            mm(vp_o[:, h, :]),
                            start=(mt == 0 and j == 0),
                            stop=(mt == NMT - 1),
                            skip_group_check=True,
                        )

        for mt in range(NMT):
            msl = ts(mt, 128)
            kps = pa_ps.tile([128, 1024], F32, tag="proj")
            for i in range(8):
                for o in range(2):
                    nc.tensor.matmul(
                        kps[:, ts(o, 512)],
                        mm(X[:, i, msl]),
                        mm(wsb["wk"][:, i, ts(o, 512)]),
                        start=(i == 0), stop=(i == 7),
                    )
                if mt < 2:
                    nc.tensor.matmul(wp, mm(wa), mm(wb), start=True,
                                     stop=True)
            if mt == 1:
                # phase-B inputs: issued after the first m-tile's matmuls so
                # phase-A waits never count these transfers.
                bg_ap = bg_d.ap()
                nc.sync.dma_start(
                    out=bg_sb[:],
                    in_=bass.AP(tensor=bg_ap.tensor, offset=0,
                                ap=[[1, 128], [128, 8]]),
                )
                nc.sync.dma_start(out=sel[:], in_=sel_d.ap())
                for i in range(8):
                    for nm in ("wq", "wg", "wo"):
                        nc.sync.dma_start(out=wsb[nm][:, i, :],
                                          in_=w_in[nm].ap()[ts(i, 128), :])
            r1 = pa_tmp.tile([128, 1024], F32, tag="r1")
            nc.scalar.activation(r1, kps, AF.Relu)
            m1 = pa_tmp.tile([128, 1024], F32, tag="m1")
            nc.vector.tensor_scalar_min(m1, kps, 0.0)
            e1 = pa_tmp.tile([128, 1024], F32, tag="e1")
            nc.scalar.activation(e1, m1, AF.Exp)
            ksb = pa_tmp.tile([128, 1024], DT, tag="ksb")
            nc.gpsimd.tensor_add(ksb, r1, e1)
            ksb_hist[mt % 2] = ksb

            vps = pa_ps.tile([128, 16, 64], F32, tag="proj")
            for i in range(8):
                for o in range(2):
                    nc.tensor.matmul(
                        vps[:, ts(o, 8), :],
                        mm(X[:, i, msl]),
                        mm(wsb["wv"][:, i, ts(o, 512)]),
                        start=(i == 0), stop=(i == 7),
                    )
                if mt < 2:
                    nc.tensor.matmul(wp, mm(wa), mm(wb), start=True,
                                     stop=True)
            vp = pa_tmp.tile([128, 16, 65], DT, tag="vp")
            nc.vector.memset(vp[:, :, 64:65], 1.0)
            nc.scalar.copy(vp[:, :, 0:64], vps[:, :, :])
            vp_hist[mt % 2] = vp

            if mt > 0:
                kv_mms(mt - 1)
        kv_mms(NMT - 1)
        warm_ps_cm.__exit__(None, None, None)
        warm_cm.__exit__(None, None, None)

        kv_sb = pa_tmp.tile([128, 8, 65], F32, tag="kv_sb", bufs=1,
                            name="kv_sb")
        for w in range(2):
            nc.vector.tensor_copy(kv_sb[:, 4 * w:4 * w + 4, :],
                                  kv_ps[w][:, :, 0:65])
        nc.sync.dma_start(out=cc_in.ap()[:, :, :], in_=kv_sb[:])

    nc.gpsimd.collective_compute(
        "AllReduce",
        mybir.AluOpType.add,
        replica_groups=[[0, 1], [2, 3], [4, 5], [6, 7]],
        ins=[cc_in.ap().opt()],
        outs=[cc_out.ap().opt()],
    )

    # ---------------- phase B ----------------
    with (
        tc.tile_pool(name="pb_tmp", bufs=2) as pb_tmp,
        tc.tile_pool(name="pb_small", bufs=1) as pb_small,
        tc.tile_pool(name="pb_qg", bufs=1) as pb_qg,
    ):
        # collective results live in the persist pool: fresh SBUF, so the
        # kvf DMA has no write-after-read wait on phase-A consumers.
        kvf = persist.tile([128, 8, 65], F32, tag="kvf")
        kvb = persist.tile([128, 8, 65], DT, tag="kvb")
        ksd = persist.tile([128, 8, 16], DT, tag="ksd")

        ps_proj_cm = tc.tile_pool(name="ps_proj", bufs=4, space="PSUM")
        ps_proj = ps_proj_cm.__enter__()

        def proj_block(p, csl, qsb, gsb, which):
            pps = ps_proj.tile([128, CHUNK], F32, tag="proj")
            wname = "wq" if which == "q" else "wg"
            for i in range(8):
                nc.tensor.matmul(
                    pps, mm(wsb[wname][:, i, ts(p, 128)]),
                    mm(X[:, i, csl]),
                    start=(i == 0), stop=(i == 7),
                )
            if which == "q":
                r1 = pb_tmp.tile([128, CHUNK], F32, tag="br1")
                nc.scalar.activation(r1, pps, AF.Relu)
                m1 = pb_tmp.tile([128, CHUNK], F32, tag="bm1")
                nc.vector.tensor_scalar_min(m1, pps, 0.0)
                e1 = pb_tmp.tile([128, CHUNK], F32, tag="be1")
                nc.scalar.activation(e1, m1, AF.Exp)
                nc.gpsimd.tensor_add(qsb[:, p, :], r1, e1)
            else:
                nc.scalar.activation(gsb[:, p, :], pps, AF.Sigmoid,
                                     bias=bg_sb[:, p:p + 1])

        # ---- pass 1: Q and G projections for ALL chunks (collective-free
        # PE work that covers the AllReduce round-trip) ----
        qsbs, gsbs = [], []
        for ch in range(NCH):
            csl = ts(ch, CHUNK)
            qsb = pb_qg.tile([128, 8, CHUNK], DT, tag=f"qsb{ch}")
            gsb = pb_qg.tile([128, 8, CHUNK], DT, tag=f"gsb{ch}")
            qsbs.append(qsb)
            gsbs.append(gsb)
            for p in range(8):
                proj_block(p, csl, qsb, gsb, "q")
            if ch == 0:
                nc.sync.dma_start(out=kvf[:], in_=cc_out.ap()[:, :, :])
            for p in range(8):
                proj_block(p, csl, qsb, gsb, "g")

        ps_proj_cm.__exit__(None, None, None)

        # ---- collective prep + scheduling gate, ALL on the Pool engine.
        # No pass-1 matmul ever waits on Pool completions, so these
        # collective-dependent ops cannot poison pass-1 counting-semaphore
        # thresholds (they did when placed on ACT or DVE). ksd2/kvb2 gain a
        # zero-valued dependency on the LAST projection block's output:
        # the Tile scheduler's readiness model ignores AllReduce latency
        # and would otherwise hoist the first collective-dependent matmul
        # right behind chunk 0's projections, head-blocking the in-order
        # PE queue on the collective.
        nc.gpsimd.tensor_copy(kvb, kvf)
        nc.gpsimd.memset(ksd[:], 0.0)
        for p in range(8):
            nc.gpsimd.tensor_scalar_mul(
                ksd[0:64, p, 2 * p:2 * p + 1],
                kvf[0:64, p, 64:65], 1.0 / SCALE)
            nc.gpsimd.tensor_scalar_mul(
                ksd[64:128, p, 2 * p + 1:2 * p + 2],
                kvf[64:128, p, 64:65], 1.0 / SCALE)
        ksd2 = persist.tile([128, 8, 16], DT, tag="ksd2")
        kvb2 = persist.tile([128, 8, 65], DT, tag="kvb2")
        zl65 = pb_small.tile([128, 65], DT, tag="zl65")
        nc.gpsimd.tensor_scalar_mul(zl65, gsbs[NCH - 1][:, 7, 0:65], 0.0)
        for p in range(8):
            nc.gpsimd.tensor_add(ksd2[:, p, :], ksd[:, p, :],
                                 zl65[:, 0:16])
            nc.gpsimd.tensor_add(kvb2[:, p, :], kvb[:, p, :], zl65)

        ps_ops_cm = tc.tile_pool(name="ps_ops", bufs=2, space="PSUM")
        ps_z_cm = tc.tile_pool(name="ps_z", bufs=2, space="PSUM")
        ps_qk_cm = tc.tile_pool(name="ps_qk", bufs=2, space="PSUM")
        ps_y_cm = tc.tile_pool(name="ps_y", bufs=2, space="PSUM")
        ps_ops = ps_ops_cm.__enter__()
        ps_z = ps_z_cm.__enter__()
        ps_qk = ps_qk_cm.__enter__()
        ps_y = ps_y_cm.__enter__()

        # ---- pass 2: attention + output projection per chunk; each
        # chunk's qk runs one chunk ahead so the z reciprocal chain (DVE)
        # hides under the previous chunk's y matmuls. The whole pass is
        # deprioritized far past pass 1 so the scheduler's ready-heap
        # always prefers projection matmuls over collective-dependent ones.
        prio_cm = tc.high_priority(offset=-1000000)
        prio_cm.__enter__()

        def qk_mms(ch):
            qkps = ps_qk.tile([16, CHUNK], F32, tag="qk")
            for p in range(8):
                nc.tensor.matmul(
                    qkps, mm(ksd2[:, p, :]), mm(qsbs[ch][:, p, :]),
                    start=(p == 0), stop=(p == 7),
                    skip_group_check=True,
                )
            return qkps

        qkps_next = qk_mms(0)
        for ch in range(NCH):
            csl = ts(ch, CHUNK)
            qsb, gsb = qsbs[ch], gsbs[ch]
            qkps = qkps_next
            zq = pb_tmp.tile([16, CHUNK], F32, tag="zq")
            nc.vector.tensor_scalar_max(zq, qkps, CLAMP)
            zr = pb_tmp.tile([16, CHUNK], F32, tag="zr")
            nc.vector.reciprocal(zr, zq)
            zqr = pb_tmp.tile([16, CHUNK], DT, tag="zqr")
            nc.vector.tensor_copy(zqr, zr)

            asb = pb_big.tile([128, 8, CHUNK], DT, tag="asb")
            for p in range(8):
                ops_ = ps_ops.tile([128, CHUNK], F32, tag="ops")
                for rr in range(2):
                    pr = slice(64 * rr, 64 * rr + 64)
                    nc.tensor.matmul(
                        ops_[pr, :], mm(kvb2[pr, p, 0:64]),
                        mm(qsb[pr, p, :]),
                        start=True, stop=True,
                    )
                zbps = ps_z.tile([128, CHUNK], F32, tag="z")
                nc.tensor.matmul(zbps, mm(sel[:, p, :]), mm(zqr),
                                 start=True, stop=True)
                t1 = pb_tmp.tile([128, CHUNK], F32, tag="bt1")
                # each mul reads at most one PSUM operand (HW restriction)
                nc.vector.tensor_mul(t1, ops_, gsb[:, p, :])
                nc.vector.tensor_mul(asb[:, p, :], t1, zbps)
                if p == 1 and ch + 1 < NCH:
                    qkps_next = qk_mms(ch + 1)

            for d in range(8):
                yps = ps_y.tile([128, CHUNK], F32, tag="y")
                for fi in range(8):
                    nc.tensor.matmul(
                        yps, mm(wsb["wo"][:, fi, ts(d, 128)]),
                        mm(asb[:, fi, :]),
                        start=(fi == 0), stop=(fi == 7),
                    )
                ysb = pb_tmp.tile([128, CHUNK], DT, tag="ysb")
                nc.scalar.copy(ysb, yps)
                nc.sync.dma_start(out=y_d.ap()[ts(d, 128), csl],
                                  in_=ysb[:])

        prio_cm.__exit__(None, None, None)
        ps_y_cm.__exit__(None, None, None)
        ps_qk_cm.__exit__(None, None, None)
        ps_z_cm.__exit__(None, None, None)
        ps_ops_cm.__exit__(None, None, None)


def _np_dt(dt_mode):
    return ml_dtypes.bfloat16 if dt_mode == "bf16" else np.float32


def prep_inputs(x, Wq, Wk, Wv, Wg, bg, Wo, dt_mode=DT_MODE):
    npdt = _np_dt(dt_mode)
    x_f = np.ascontiguousarray(np.asarray(x, np.float32).reshape(B * N, DIM))
    w_t = {}
    for nm, W in (("wq", Wq), ("wk", Wk), ("wv", Wv), ("wg", Wg)):
        w_t[nm] = np.ascontiguousarray(
            np.asarray(W, np.float32).T).astype(npdt)
    w_t["wo"] = np.ascontiguousarray(
        np.asarray(Wo, np.float32).T).astype(npdt)
    bg_f = np.ascontiguousarray(np.asarray(bg, np.float32))
    in_maps = []
    for c in range(N_CORES):
        xt_c = np.ascontiguousarray(
            x_f[c * TPC:(c + 1) * TPC].T).astype(npdt)
        m = {"xt": xt_c, "bg": bg_f}
        m.update(w_t)
        in_maps.append(m)
    return in_maps


def unshard_output(y_parts):
    out = np.empty((B * N, DIM), np.float32)
    for c in range(N_CORES):
        out[c * TPC:(c + 1) * TPC] = np.asarray(y_parts[c], np.float32).T
    return out.reshape(B, N, DIM)


def get_nc(dt_mode=DT_MODE):
    key = ("nc", dt_mode)
    if key not in _CACHE:
        _CACHE[key] = _build(dt_mode)
    return _CACHE[key]


def kernel(x, Wq, Wk, Wv, Wg, bg, Wo):
    from concourse.bass_utils import run_bass_kernel_spmd

    nc = get_nc()
    in_maps = prep_inputs(x, Wq, Wk, Wv, Wg, bg, Wo)
    res = run_bass_kernel_spmd(nc, in_maps, core_ids=list(range(N_CORES)))
    return unshard_output([res.results[c]["y"] for c in range(N_CORES)])


# revision 46
# speedup vs baseline: 1.1105x; 1.0452x over previous
"""Trainium2 Bass kernel for nn_GatedAttention (linear attention with sigmoid
gate).

Strategy: shard the 16384 token rows across 8 cores (2048 each; cores 2b,2b+1
hold batch b). Per core, two phases:
  A: K,V projections (token-major) + per-head kv' = K^T [V|1] accumulated in
     persistent PSUM tiles over all local tokens (the ones column folds k_sum
     into kv'). kv matmuls run one m-tile behind the projections so the elu
     chain never stalls the PE.
  -- pairwise AllReduce of kv' between the two cores sharing a batch --
  B: Q,G projections (feature-major), out^T = kv'^T @ Q per head, normalizer
     z = SCALE/max(q.k_sum,eps) applied via tiny selector matmuls, gate, and
     the final output projection, all feature-major.
DMAs are issued in consumer order (X/wk/wv before phase A, wq/wg/wo/bg/sel
after the first m-tile, collective output after ch0's Q matmuls) so counting
semaphores never serialize the PE behind unrelated transfers.
Host transposes x to feature-major and pre-transposes weights; output returns
feature-major bf16 per-core slabs that the host transposes back.
"""
import sys

sys.path.insert(0, "/opt/trn_rl_repo")

import numpy as np
import ml_dtypes

B, N, DIM = 4, 4096, 1024
HEADS, DH = 16, 64
SCALE = DH ** -0.5
N_CORES = 8
TPC = B * N // N_CORES      # 2048 tokens per core
NMT = TPC // 128            # 16 m-tiles (phase A)
CHUNK = 512
NCH = TPC // CHUNK          # 4 chunks (phase B)
CLAMP = 1e-6 / SCALE

DT_MODE = "bf16"            # "bf16" | "f32r" | "f32"

_CACHE = {}


def _build(dt_mode=DT_MODE, reps=1):
    import concourse.bacc as bacc
    import concourse.bass as bass
    import concourse.tile as tile
    from concourse import mybir

    AF = mybir.ActivationFunctionType
    F32 = mybir.dt.float32
    F8 = mybir.dt.float8e4
    DR = mybir.MatmulPerfMode.DoubleRow
    DT = mybir.dt.bfloat16 if dt_mode == "bf16" else mybir.dt.float32

    def mm(ap):
        return ap.bitcast(mybir.dt.float32r) if dt_mode == "f32r" else ap

    ts = bass.ts

    nc = bacc.Bacc("TRN2", target_bir_lowering=False, debug=False,
                   num_devices=N_CORES)
    xt = nc.dram_tensor("xt", [DIM, TPC], DT, kind="ExternalInput")
    x8_d = nc.dram_tensor("x8", [DIM, TPC], F8, kind="ExternalInput")
    w_in = {}
    for nm in ("wv", "wg", "wo"):
        w_in[nm] = nc.dram_tensor(nm, [DIM, DIM], DT, kind="ExternalInput")
    for nm in ("wk8", "wq8"):
        w_in[nm] = nc.dram_tensor(nm, [DIM, DIM], F8, kind="ExternalInput")
    bg_d = nc.dram_tensor("bg", [DIM], F32, kind="ExternalInput")
    y_d = nc.dram_tensor("y", [DIM, TPC], DT, kind="ExternalOutput")
    cc_in = nc.dram_tensor("cc_in", [128, 8, 65], F32)
    cc_out = nc.dram_tensor("cc_out", [128, 8, 65], F32)

    with tile.TileContext(nc, num_cores=N_CORES) as tc:
        with (
            tc.tile_pool(name="persist", bufs=1) as persist,
            tc.tile_pool(name="pb_big", bufs=2) as pb_big,
        ):
            X = persist.tile([128, 8, TPC], DT, tag="x")
            X8 = persist.tile([128, 8, TPC], F8, tag="x8")
            wsb = {}
            for nm in ("wg", "wo"):
                wsb[nm] = persist.tile([128, 8, DIM], DT, tag=nm, name=nm)
            wsb["wq8"] = persist.tile([128, 8, DIM], F8, tag="wq8",
                                      name="wq8")
            bg_sb = persist.tile([128, 8], F32, tag="bg")
            sel_np = np.zeros((16, 8, 128), _np_dt(dt_mode))
            for p in range(8):
                sel_np[2 * p, p, 0:64] = 1.0
                sel_np[2 * p + 1, p, 64:128] = 1.0
            sel_d = nc.inline_tensor(sel_np, name="sel_const")
            sel = persist.tile([16, 8, 128], DT, tag="sel")

            for _rep in range(reps):
                _phases(nc, tc, bass, mybir, AF, F32, DT, mm, ts, X, wsb,
                        bg_sb, sel, sel_d, w_in, xt, bg_d, cc_in, cc_out, y_d,
                        tc_pools=(persist, pb_big), X8=X8, x8_d=x8_d,
                        F8=F8, DR=DR)
    nc.compile()
    return nc


def _phases(nc, tc, bass, mybir, AF, F32, DT, mm, ts, X, wsb, bg_sb, sel,
            sel_d, w_in, xt, bg_d, cc_in, cc_out, y_d, tc_pools, X8, x8_d,
            F8, DR):
    persist, pb_big = tc_pools
    # ---------------- phase A ----------------
    with (
        tc.tile_pool(name="pa_w", bufs=1) as pa_w,
        tc.tile_pool(name="pa_tmp", bufs=2) as pa_tmp,
        tc.tile_pool(name="pa_ps", bufs=2, space="PSUM") as pa_ps,
        tc.tile_pool(name="kv_ps", bufs=1, space="PSUM") as kv_pool,
    ):
        wsb["wk8"] = pa_w.tile([128, 8, DIM], F8, tag="wk8", name="wk8")
        wsb["wv"] = pa_w.tile([128, 8, DIM], DT, tag="wv", name="wv")
        # consumer-ordered DMA issue: phase A inputs first, interleaved by
        # contraction chunk so the first m-tile can start ASAP (X/wv are
        # only needed once the first V-projection starts, ~1.7us after K).
        for i in range(8):
            nc.sync.dma_start(out=X8[:, i, :], in_=x8_d.ap()[ts(i, 128), :])
            nc.sync.dma_start(out=wsb["wk8"][:, i, :],
                              in_=w_in["wk8"].ap()[ts(i, 128), :])
        for i in range(8):
            nc.sync.dma_start(out=X[:, i, :], in_=xt.ap()[ts(i, 128), :])
            nc.sync.dma_start(out=wsb["wv"][:, i, :],
                              in_=w_in["wv"].ap()[ts(i, 128), :])

        # HAM warm-up: the PE clock sits at 1.2GHz until ~3.4us of sustained
        # matmul activity, and the input DMAs take ~12us to land AND pace
        # the first two m-tiles. Dummy matmuls on zeroed scratch fill the
        # initial window, and more are interleaved into the DMA-paced
        # m-tiles (below) so the activity window never sees an idle gap.
        warm_cm = tc.tile_pool(name="warm", bufs=1)
        warm_pool = warm_cm.__enter__()
        warm_ps_cm = tc.tile_pool(name="warm_ps", bufs=1, space="PSUM")
        warm_ps = warm_ps_cm.__enter__()
        wa = warm_pool.tile([128, 128], DT, tag="wa")
        wb = warm_pool.tile([128, 512], DT, tag="wb")
        nc.vector.memset(wa[:], 0.0)
        nc.vector.memset(wb[:], 0.0)
        wp = warm_ps.tile([128, 512], F32, tag="wp")
        for _ in range(14):
            nc.tensor.matmul(wp, mm(wa), mm(wb), start=True, stop=True)

        # persistent PSUM accumulators for kv': 2 tiles x [128, 4, 128]
        # (tile w holds heads 8w..8w+7: slice j rows 0:64 = head 8w+2j,
        #  rows 64:128 = head 8w+2j+1; j-slices padded to 128 floats so a
        #  matmul output never crosses a PSUM bank boundary)
        kv_ps = [kv_pool.tile([128, 4, 128], F32, tag=f"kv{w}",
                              name=f"kv{w}") for w in range(2)]
        ksb_hist = [None, None]
        vp_hist = [None, None]

        def kv_mms(mt):
            ksb_o = ksb_hist[mt % 2]
            vp_o = vp_hist[mt % 2]
            for w in range(2):
                for j in range(4):
                    for c in range(2):
                        h = 8 * w + 2 * j + c
                        # start only on the FIRST matmul touching this bank's
                        # partition plane: start_tensor_calc marks the whole
                        # 2KB zero-region pending, so a second start=True in
                        # the same bank would re-poison already-written
                        # slices and turn later accumulates into overwrites.
                        nc.tensor.matmul(
                            kv_ps[w][64 * c:64 * c + 64, j, 0:65],
                            mm(ksb_o[:, ts(h, 64)]),
                            mm(vp_o[:, h, :]),
                            start=(mt == 0 and j == 0),
                            stop=(mt == NMT - 1),
                            skip_group_check=True,
                        )

        for mt in range(NMT):
            msl = ts(mt, 128)
            kps = pa_ps.tile([128, 1024], F32, tag="proj")
            # K projection in fp8 DoubleRow: contraction pairs of 128-chunks
            # (effective K=256 per matmul, ~1.4-2x PE throughput); fp8 on
            # Q/K is accuracy-safe because the normalizer z cancels most of
            # the quantization error (measured rel 0.0069 vs 0.02 budget).
            for j in range(4):
                for o in range(2):
                    nc.tensor.matmul(
                        kps[:, ts(o, 512)],
                        X8[:, 2 * j:2 * j + 2, msl],
                        wsb["wk8"][:, 2 * j:2 * j + 2, ts(o, 512)],
                        start=(j == 0), stop=(j == 3),
                        perf_mode=DR,
                    )
                if mt < 2:
                    nc.tensor.matmul(wp, mm(wa), mm(wb), start=True,
                                     stop=True)
            if mt == 1:
                # phase-B inputs: issued after the first m-tile's matmuls so
                # phase-A waits never count these transfers.
                bg_ap = bg_d.ap()
                nc.sync.dma_start(
                    out=bg_sb[:],
                    in_=bass.AP(tensor=bg_ap.tensor, offset=0,
                                ap=[[1, 128], [128, 8]]),
                )
                nc.sync.dma_start(out=sel[:], in_=sel_d.ap())
                for i in range(8):
                    for nm in ("wq8", "wg", "wo"):
                        nc.sync.dma_start(out=wsb[nm][:, i, :],
                                          in_=w_in[nm].ap()[ts(i, 128), :])
            r1 = pa_tmp.tile([128, 1024], F32, tag="r1")
            nc.scalar.activation(r1, kps, AF.Relu)
            m1 = pa_tmp.tile([128, 1024], F32, tag="m1")
            nc.vector.tensor_scalar_min(m1, kps, 0.0)
            e1 = pa_tmp.tile([128, 1024], F32, tag="e1")
            nc.scalar.activation(e1, m1, AF.Exp)
            ksb = pa_tmp.tile([128, 1024], DT, tag="ksb")
            nc.gpsimd.tensor_add(ksb, r1, e1)
            ksb_hist[mt % 2] = ksb

            vps = pa_ps.tile([128, 16, 64], F32, tag="proj")
            for i in range(8):
                for o in range(2):
                    nc.tensor.matmul(
                        vps[:, ts(o, 8), :],
                        mm(X[:, i, msl]),
                        mm(wsb["wv"][:, i, ts(o, 512)]),
                        start=(i == 0), stop=(i == 7),
                    )
                if mt < 2:
                    nc.tensor.matmul(wp, mm(wa), mm(wb), start=True,
                                     stop=True)
            vp = pa_tmp.tile([128, 16, 65], DT, tag="vp")
            nc.vector.memset(vp[:, :, 64:65], 1.0)
            nc.scalar.copy(vp[:, :, 0:64], vps[:, :, :])
            vp_hist[mt % 2] = vp

            if mt > 0:
                kv_mms(mt - 1)
        kv_mms(NMT - 1)
        warm_ps_cm.__exit__(None, None, None)
        warm_cm.__exit__(None, None, None)

        kv_sb = pa_tmp.tile([128, 8, 65], F32, tag="kv_sb", bufs=1,
                            name="kv_sb")
        for w in range(2):
            nc.vector.tensor_copy(kv_sb[:, 4 * w:4 * w + 4, :],
                                  kv_ps[w][:, :, 0:65])
        nc.sync.dma_start(out=cc_in.ap()[:, :, :], in_=kv_sb[:])

    nc.gpsimd.collective_compute(
        "AllReduce",
        mybir.AluOpType.add,
        replica_groups=[[0, 1], [2, 3], [4, 5], [6, 7]],
        ins=[cc_in.ap().opt()],
        outs=[cc_out.ap().opt()],
    )

    # ---------------- phase B ----------------
    with (
        tc.tile_pool(name="pb_tmp", bufs=2) as pb_tmp,
        tc.tile_pool(name="pb_small", bufs=1) as pb_small,
        tc.tile_pool(name="pb_qg", bufs=1) as pb_qg,
    ):
        # collective results live in the persist pool: fresh SBUF, so the
        # kvf DMA has no write-after-read wait on phase-A consumers.
        kvf = persist.tile([128, 8, 65], F32, tag="kvf")
        kvb = persist.tile([128, 8, 65], DT, tag="kvb")
        ksd = persist.tile([128, 8, 16], DT, tag="ksd")

        ps_proj_cm = tc.tile_pool(name="ps_proj", bufs=4, space="PSUM")
        ps_proj = ps_proj_cm.__enter__()

        def proj_block(p, csl, qsb, gsb, which):
            pps = ps_proj.tile([128, CHUNK], F32, tag="proj")
            if which == "q":
                # fp8 DoubleRow (see K projection note)
                for j in range(4):
                    nc.tensor.matmul(
                        pps, wsb["wq8"][:, 2 * j:2 * j + 2, ts(p, 128)],
                        X8[:, 2 * j:2 * j + 2, csl],
                        start=(j == 0), stop=(j == 3),
                        perf_mode=DR,
                    )
            else:
                for i in range(8):
                    nc.tensor.matmul(
                        pps, mm(wsb["wg"][:, i, ts(p, 128)]),
                        mm(X[:, i, csl]),
                        start=(i == 0), stop=(i == 7),
                    )
            if which == "q":
                r1 = pb_tmp.tile([128, CHUNK], F32, tag="br1")
                nc.scalar.activation(r1, pps, AF.Relu)
                m1 = pb_tmp.tile([128, CHUNK], F32, tag="bm1")
                nc.vector.tensor_scalar_min(m1, pps, 0.0)
                e1 = pb_tmp.tile([128, CHUNK], F32, tag="be1")
                nc.scalar.activation(e1, m1, AF.Exp)
                nc.gpsimd.tensor_add(qsb[:, p, :], r1, e1)
            else:
                nc.scalar.activation(gsb[:, p, :], pps, AF.Sigmoid,
                                     bias=bg_sb[:, p:p + 1])

        # ---- pass 1: Q and G projections for ALL chunks (collective-free
        # PE work that covers the AllReduce round-trip) ----
        qsbs, gsbs = [], []
        for ch in range(NCH):
            csl = ts(ch, CHUNK)
            qsb = pb_qg.tile([128, 8, CHUNK], DT, tag=f"qsb{ch}")
            gsb = pb_qg.tile([128, 8, CHUNK], DT, tag=f"gsb{ch}")
            qsbs.append(qsb)
            gsbs.append(gsb)
            for p in range(8):
                proj_block(p, csl, qsb, gsb, "q")
            if ch == 0:
                nc.sync.dma_start(out=kvf[:], in_=cc_out.ap()[:, :, :])
            for p in range(8):
                proj_block(p, csl, qsb, gsb, "g")

        ps_proj_cm.__exit__(None, None, None)

        # ---- collective prep + scheduling gate, ALL on the Pool engine.
        # No pass-1 matmul ever waits on Pool completions, so these
        # collective-dependent ops cannot poison pass-1 counting-semaphore
        # thresholds (they did when placed on ACT or DVE). ksd2/kvb2 gain a
        # zero-valued dependency on the LAST projection block's output:
        # the Tile scheduler's readiness model ignores AllReduce latency
        # and would otherwise hoist the first collective-dependent matmul
        # right behind chunk 0's projections, head-blocking the in-order
        # PE queue on the collective.
        nc.gpsimd.tensor_copy(kvb, kvf)
        nc.gpsimd.memset(ksd[:], 0.0)
        for p in range(8):
            nc.gpsimd.tensor_scalar_mul(
                ksd[0:64, p, 2 * p:2 * p + 1],
                kvf[0:64, p, 64:65], 1.0 / SCALE)
            nc.gpsimd.tensor_scalar_mul(
                ksd[64:128, p, 2 * p + 1:2 * p + 2],
                kvf[64:128, p, 64:65], 1.0 / SCALE)
        ksd2 = persist.tile([128, 8, 16], DT, tag="ksd2")
        kvb2 = persist.tile([128, 8, 65], DT, tag="kvb2")
        zl65 = pb_small.tile([128, 65], DT, tag="zl65")
        nc.gpsimd.tensor_scalar_mul(zl65, gsbs[NCH - 1][:, 7, 0:65], 0.0)
        for p in range(8):
            nc.gpsimd.tensor_add(ksd2[:, p, :], ksd[:, p, :],
                                 zl65[:, 0:16])
            nc.gpsimd.tensor_add(kvb2[:, p, :], kvb[:, p, :], zl65)

        ps_ops_cm = tc.tile_pool(name="ps_ops", bufs=2, space="PSUM")
        ps_z_cm = tc.tile_pool(name="ps_z", bufs=2, space="PSUM")
        ps_qk_cm = tc.tile_pool(name="ps_qk", bufs=2, space="PSUM")
        ps_y_cm = tc.tile_pool(name="ps_y", bufs=2, space="PSUM")
        ps_ops = ps_ops_cm.__enter__()
        ps_z = ps_z_cm.__enter__()
        ps_qk = ps_qk_cm.__enter__()
        ps_y = ps_y_cm.__enter__()

        # ---- pass 2: attention + output projection per chunk; each
        # chunk's qk runs one chunk ahead so the z reciprocal chain (DVE)
        # hides under the previous chunk's y matmuls. The whole pass is
        # deprioritized far past pass 1 so the scheduler's ready-heap
        # always prefers projection matmuls over collective-dependent ones.
        prio_cm = tc.high_priority(offset=-1000000)
        prio_cm.__enter__()

        def qk_mms(ch):
            qkps = ps_qk.tile([16, CHUNK], F32, tag="qk")
            for p in range(8):
                nc.tensor.matmul(
                    qkps, mm(ksd2[:, p, :]), mm(qsbs[ch][:, p, :]),
                    start=(p == 0), stop=(p == 7),
                    skip_group_check=True,
                )
            return qkps

        qkps_next = qk_mms(0)
        for ch in range(NCH):
            csl = ts(ch, CHUNK)
            qsb, gsb = qsbs[ch], gsbs[ch]
            qkps = qkps_next
            zq = pb_tmp.tile([16, CHUNK], F32, tag="zq")
            nc.vector.tensor_scalar_max(zq, qkps, CLAMP)
            zr = pb_tmp.tile([16, CHUNK], F32, tag="zr")
            nc.vector.reciprocal(zr, zq)
            zqr = pb_tmp.tile([16, CHUNK], DT, tag="zqr")
            nc.vector.tensor_copy(zqr, zr)

            asb = pb_big.tile([128, 8, CHUNK], DT, tag="asb")
            for p in range(8):
                ops_ = ps_ops.tile([128, CHUNK], F32, tag="ops")
                for rr in range(2):
                    pr = slice(64 * rr, 64 * rr + 64)
                    nc.tensor.matmul(
                        ops_[pr, :], mm(kvb2[pr, p, 0:64]),
                        mm(qsb[pr, p, :]),
                        start=True, stop=True,
                    )
                zbps = ps_z.tile([128, CHUNK], F32, tag="z")
                nc.tensor.matmul(zbps, mm(sel[:, p, :]), mm(zqr),
                                 start=True, stop=True)
                t1 = pb_tmp.tile([128, CHUNK], F32, tag="bt1")
                # each mul reads at most one PSUM operand (HW restriction)
                nc.vector.tensor_mul(t1, ops_, gsb[:, p, :])
                nc.vector.tensor_mul(asb[:, p, :], t1, zbps)
                if p == 1 and ch + 1 < NCH:
                    qkps_next = qk_mms(ch + 1)

            for d in range(8):
                yps = ps_y.tile([128, CHUNK], F32, tag="y")
                for fi in range(8):
                    nc.tensor.matmul(
                        yps, mm(wsb["wo"][:, fi, ts(d, 128)]),
                        mm(asb[:, fi, :]),
                        start=(fi == 0), stop=(fi == 7),
                    )
                ysb = pb_tmp.tile([128, CHUNK], DT, tag="ysb")
                nc.scalar.copy(ysb, yps)
                nc.sync.dma_start(out=y_d.ap()[ts(d, 128), csl],
                                  in_=ysb[:])

        prio_cm.__exit__(None, None, None)
        ps_y_cm.__exit__(None, None, None)
        ps_qk_cm.__exit__(None, None, None)
        ps_z_cm.__exit__(None, None, None)
        ps_ops_cm.__exit__(None, None, None)


def _np_dt(dt_mode):
    return ml_dtypes.bfloat16 if dt_mode == "bf16" else np.float32


def prep_inputs(x, Wq, Wk, Wv, Wg, bg, Wo, dt_mode=DT_MODE):
    npdt = _np_dt(dt_mode)
    f8 = ml_dtypes.float8_e4m3
    x_f = np.ascontiguousarray(np.asarray(x, np.float32).reshape(B * N, DIM))
    w_t = {}
    for nm, W in (("wv", Wv), ("wg", Wg)):
        w_t[nm] = np.ascontiguousarray(
            np.asarray(W, np.float32).T).astype(npdt)
    for nm, W in (("wq8", Wq), ("wk8", Wk)):
        w_t[nm] = np.ascontiguousarray(
            np.asarray(W, np.float32).T).astype(f8)
    w_t["wo"] = np.ascontiguousarray(
        np.asarray(Wo, np.float32).T).astype(npdt)
    bg_f = np.ascontiguousarray(np.asarray(bg, np.float32))
    in_maps = []
    for c in range(N_CORES):
        xt_T = np.ascontiguousarray(x_f[c * TPC:(c + 1) * TPC].T)
        m = {"xt": xt_T.astype(npdt), "x8": xt_T.astype(f8), "bg": bg_f}
        m.update(w_t)
        in_maps.append(m)
    return in_maps


def unshard_output(y_parts):
    out = np.empty((B * N, DIM), np.float32)
    for c in range(N_CORES):
        out[c * TPC:(c + 1) * TPC] = np.asarray(y_parts[c], np.float32).T
    return out.reshape(B, N, DIM)


def get_nc(dt_mode=DT_MODE):
    key = ("nc", dt_mode)
    if key not in _CACHE:
        _CACHE[key] = _build(dt_mode)
    return _CACHE[key]


def kernel(x, Wq, Wk, Wv, Wg, bg, Wo):
    from concourse.bass_utils import run_bass_kernel_spmd

    nc = get_nc()
    in_maps = prep_inputs(x, Wq, Wk, Wv, Wg, bg, Wo)
    res = run_bass_kernel_spmd(nc, in_maps, core_ids=list(range(N_CORES)))
    return unshard_output([res.results[c]["y"] for c in range(N_CORES)])


# revision 48
# speedup vs baseline: 1.1220x; 1.0103x over previous
"""Trainium2 Bass kernel for nn_GatedAttention (linear attention with sigmoid
gate).

Strategy: shard the 16384 token rows across 8 cores (2048 each; cores 2b,2b+1
hold batch b). Per core, two phases:
  A: K,V projections (token-major) + per-head kv' = K^T [V|1] accumulated in
     persistent PSUM tiles over all local tokens (the ones column folds k_sum
     into kv'). kv matmuls run one m-tile behind the projections so the elu
     chain never stalls the PE.
  -- pairwise AllReduce of kv' between the two cores sharing a batch --
  B: Q,G projections (feature-major), out^T = kv'^T @ Q per head, normalizer
     z = SCALE/max(q.k_sum,eps) applied via tiny selector matmuls, gate, and
     the final output projection, all feature-major.
DMAs are issued in consumer order (X/wk/wv before phase A, wq/wg/wo/bg/sel
after the first m-tile, collective output after ch0's Q matmuls) so counting
semaphores never serialize the PE behind unrelated transfers.
Host transposes x to feature-major and pre-transposes weights; output returns
feature-major bf16 per-core slabs that the host transposes back.
"""
import sys

sys.path.insert(0, "/opt/trn_rl_repo")

import numpy as np
import ml_dtypes

B, N, DIM = 4, 4096, 1024
HEADS, DH = 16, 64
SCALE = DH ** -0.5
N_CORES = 8
TPC = B * N // N_CORES      # 2048 tokens per core
NMT = TPC // 128            # 16 m-tiles (phase A)
CHUNK = 512
NCH = TPC // CHUNK          # 4 chunks (phase B)
CLAMP = 1e-6 / SCALE

DT_MODE = "bf16"            # "bf16" | "f32r" | "f32"

_CACHE = {}


def _build(dt_mode=DT_MODE, reps=1):
    import concourse.bacc as bacc
    import concourse.bass as bass
    import concourse.tile as tile
    from concourse import mybir

    AF = mybir.ActivationFunctionType
    F32 = mybir.dt.float32
    F8 = mybir.dt.float8e4
    DR = mybir.MatmulPerfMode.DoubleRow
    DT = mybir.dt.bfloat16 if dt_mode == "bf16" else mybir.dt.float32

    def mm(ap):
        return ap.bitcast(mybir.dt.float32r) if dt_mode == "f32r" else ap

    ts = bass.ts

    nc = bacc.Bacc("TRN2", target_bir_lowering=False, debug=False,
                   num_devices=N_CORES)
    xt = nc.dram_tensor("xt", [DIM, TPC], DT, kind="ExternalInput")
    x8_d = nc.dram_tensor("x8", [DIM, TPC], F8, kind="ExternalInput")
    w_in = {}
    for nm in ("wv", "wo"):
        w_in[nm] = nc.dram_tensor(nm, [DIM, DIM], DT, kind="ExternalInput")
    for nm in ("wk8", "wq8", "wg8"):
        w_in[nm] = nc.dram_tensor(nm, [DIM, DIM], F8, kind="ExternalInput")
    bg_d = nc.dram_tensor("bg", [DIM], F32, kind="ExternalInput")
    y_d = nc.dram_tensor("y", [DIM, TPC], DT, kind="ExternalOutput")
    cc_in = nc.dram_tensor("cc_in", [128, 8, 65], F32)
    cc_out = nc.dram_tensor("cc_out", [128, 8, 65], F32)

    with tile.TileContext(nc, num_cores=N_CORES) as tc:
        with (
            tc.tile_pool(name="persist", bufs=1) as persist,
            tc.tile_pool(name="pb_big", bufs=2) as pb_big,
        ):
            X = persist.tile([128, 8, TPC], DT, tag="x")
            X8 = persist.tile([128, 8, TPC], F8, tag="x8")
            wsb = {}
            wsb["wo"] = persist.tile([128, 8, DIM], DT, tag="wo", name="wo")
            for nm in ("wq8", "wg8"):
                wsb[nm] = persist.tile([128, 8, DIM], F8, tag=nm, name=nm)
            bg_sb = persist.tile([128, 8], F32, tag="bg")
            sel_np = np.zeros((16, 8, 128), _np_dt(dt_mode))
            for p in range(8):
                sel_np[2 * p, p, 0:64] = 1.0
                sel_np[2 * p + 1, p, 64:128] = 1.0
            sel_d = nc.inline_tensor(sel_np, name="sel_const")
            sel = persist.tile([16, 8, 128], DT, tag="sel")

            for _rep in range(reps):
                _phases(nc, tc, bass, mybir, AF, F32, DT, mm, ts, X, wsb,
                        bg_sb, sel, sel_d, w_in, xt, bg_d, cc_in, cc_out, y_d,
                        tc_pools=(persist, pb_big), X8=X8, x8_d=x8_d,
                        F8=F8, DR=DR)
    nc.compile()
    return nc


def _phases(nc, tc, bass, mybir, AF, F32, DT, mm, ts, X, wsb, bg_sb, sel,
            sel_d, w_in, xt, bg_d, cc_in, cc_out, y_d, tc_pools, X8, x8_d,
            F8, DR):
    persist, pb_big = tc_pools
    # ---------------- phase A ----------------
    with (
        tc.tile_pool(name="pa_w", bufs=1) as pa_w,
        tc.tile_pool(name="pa_tmp", bufs=2) as pa_tmp,
        tc.tile_pool(name="pa_ps", bufs=2, space="PSUM") as pa_ps,
        tc.tile_pool(name="kv_ps", bufs=1, space="PSUM") as kv_pool,
    ):
        wsb["wk8"] = pa_w.tile([128, 8, DIM], F8, tag="wk8", name="wk8")
        wsb["wv"] = pa_w.tile([128, 8, DIM], DT, tag="wv", name="wv")
        # consumer-ordered DMA issue: phase A inputs first, interleaved by
        # contraction chunk so the first m-tile can start ASAP (X/wv are
        # only needed once the first V-projection starts, ~1.7us after K).
        for i in range(8):
            nc.sync.dma_start(out=X8[:, i, :], in_=x8_d.ap()[ts(i, 128), :])
            nc.sync.dma_start(out=wsb["wk8"][:, i, :],
                              in_=w_in["wk8"].ap()[ts(i, 128), :])
        for i in range(8):
            nc.sync.dma_start(out=X[:, i, :], in_=xt.ap()[ts(i, 128), :])
            nc.sync.dma_start(out=wsb["wv"][:, i, :],
                              in_=w_in["wv"].ap()[ts(i, 128), :])

        # HAM warm-up: the PE clock sits at 1.2GHz until ~3.4us of sustained
        # matmul activity, and the input DMAs take ~12us to land AND pace
        # the first two m-tiles. Dummy matmuls on zeroed scratch fill the
        # initial window, and more are interleaved into the DMA-paced
        # m-tiles (below) so the activity window never sees an idle gap.
        warm_cm = tc.tile_pool(name="warm", bufs=1)
        warm_pool = warm_cm.__enter__()
        warm_ps_cm = tc.tile_pool(name="warm_ps", bufs=1, space="PSUM")
        warm_ps = warm_ps_cm.__enter__()
        wa = warm_pool.tile([128, 128], DT, tag="wa")
        wb = warm_pool.tile([128, 512], DT, tag="wb")
        nc.vector.memset(wa[:], 0.0)
        nc.vector.memset(wb[:], 0.0)
        wp = warm_ps.tile([128, 512], F32, tag="wp")
        for _ in range(14):
            nc.tensor.matmul(wp, mm(wa), mm(wb), start=True, stop=True)

        # persistent PSUM accumulators for kv': 2 tiles x [128, 4, 128]
        # (tile w holds heads 8w..8w+7: slice j rows 0:64 = head 8w+2j,
        #  rows 64:128 = head 8w+2j+1; j-slices padded to 128 floats so a
        #  matmul output never crosses a PSUM bank boundary)
        kv_ps = [kv_pool.tile([128, 4, 128], F32, tag=f"kv{w}",
                              name=f"kv{w}") for w in range(2)]
        ksb_hist = [None, None]
        vp_hist = [None, None]

        def kv_mms(mt):
            ksb_o = ksb_hist[mt % 2]
            vp_o = vp_hist[mt % 2]
            for w in range(2):
                for j in range(4):
                    for c in range(2):
                        h = 8 * w + 2 * j + c
                        # start only on the FIRST matmul touching this bank's
                        # partition plane: start_tensor_calc marks the whole
                        # 2KB zero-region pending, so a second start=True in
                        # the same bank would re-poison already-written
                        # slices and turn later accumulates into overwrites.
                        nc.tensor.matmul(
                            kv_ps[w][64 * c:64 * c + 64, j, 0:65],
                            mm(ksb_o[:, ts(h, 64)]),
                            mm(vp_o[:, h, :]),
                            start=(mt == 0 and j == 0),
                            stop=(mt == NMT - 1),
                            skip_group_check=True,
                        )

        for mt in range(NMT):
            msl = ts(mt, 128)
            kps = pa_ps.tile([128, 1024], F32, tag="proj")
            # K projection in fp8 DoubleRow: contraction pairs of 128-chunks
            # (effective K=256 per matmul, ~1.4-2x PE throughput); fp8 on
            # Q/K is accuracy-safe because the normalizer z cancels most of
            # the quantization error (measured rel 0.0069 vs 0.02 budget).
            for j in range(4):
                for o in range(2):
                    nc.tensor.matmul(
                        kps[:, ts(o, 512)],
                        X8[:, 2 * j:2 * j + 2, msl],
                        wsb["wk8"][:, 2 * j:2 * j + 2, ts(o, 512)],
                        start=(j == 0), stop=(j == 3),
                        perf_mode=DR,
                    )
                if mt < 2:
                    nc.tensor.matmul(wp, mm(wa), mm(wb), start=True,
                                     stop=True)
            if mt == 1:
                # phase-B inputs: issued after the first m-tile's matmuls so
                # phase-A waits never count these transfers.
                bg_ap = bg_d.ap()
                nc.sync.dma_start(
                    out=bg_sb[:],
                    in_=bass.AP(tensor=bg_ap.tensor, offset=0,
                                ap=[[1, 128], [128, 8]]),
                )
                nc.sync.dma_start(out=sel[:], in_=sel_d.ap())
                for i in range(8):
                    for nm in ("wq8", "wg8", "wo"):
                        nc.sync.dma_start(out=wsb[nm][:, i, :],
                                          in_=w_in[nm].ap()[ts(i, 128), :])
            r1 = pa_tmp.tile([128, 1024], F32, tag="r1")
            nc.scalar.activation(r1, kps, AF.Relu)
            m1 = pa_tmp.tile([128, 1024], F32, tag="m1")
            nc.vector.tensor_scalar_min(m1, kps, 0.0)
            e1 = pa_tmp.tile([128, 1024], F32, tag="e1")
            nc.scalar.activation(e1, m1, AF.Exp)
            ksb = pa_tmp.tile([128, 1024], DT, tag="ksb")
            nc.gpsimd.tensor_add(ksb, r1, e1)
            ksb_hist[mt % 2] = ksb

            vps = pa_ps.tile([128, 16, 64], F32, tag="proj")
            for i in range(8):
                for o in range(2):
                    nc.tensor.matmul(
                        vps[:, ts(o, 8), :],
                        mm(X[:, i, msl]),
                        mm(wsb["wv"][:, i, ts(o, 512)]),
                        start=(i == 0), stop=(i == 7),
                    )
                if mt < 2:
                    nc.tensor.matmul(wp, mm(wa), mm(wb), start=True,
                                     stop=True)
            vp = pa_tmp.tile([128, 16, 65], DT, tag="vp")
            nc.vector.memset(vp[:, :, 64:65], 1.0)
            nc.scalar.copy(vp[:, :, 0:64], vps[:, :, :])
            vp_hist[mt % 2] = vp

            if mt > 0:
                kv_mms(mt - 1)
        kv_mms(NMT - 1)
        warm_ps_cm.__exit__(None, None, None)
        warm_cm.__exit__(None, None, None)

        kv_sb = pa_tmp.tile([128, 8, 65], F32, tag="kv_sb", bufs=1,
                            name="kv_sb")
        for w in range(2):
            nc.vector.tensor_copy(kv_sb[:, 4 * w:4 * w + 4, :],
                                  kv_ps[w][:, :, 0:65])
        nc.sync.dma_start(out=cc_in.ap()[:, :, :], in_=kv_sb[:])

    nc.gpsimd.collective_compute(
        "AllReduce",
        mybir.AluOpType.add,
        replica_groups=[[0, 1], [2, 3], [4, 5], [6, 7]],
        ins=[cc_in.ap().opt()],
        outs=[cc_out.ap().opt()],
    )

    # ---------------- phase B ----------------
    with (
        tc.tile_pool(name="pb_tmp", bufs=2) as pb_tmp,
        tc.tile_pool(name="pb_small", bufs=1) as pb_small,
        tc.tile_pool(name="pb_qg", bufs=1) as pb_qg,
    ):
        # collective results live in the persist pool: fresh SBUF, so the
        # kvf DMA has no write-after-read wait on phase-A consumers.
        kvf = persist.tile([128, 8, 65], F32, tag="kvf")
        kvb = persist.tile([128, 8, 65], DT, tag="kvb")
        ksd = persist.tile([128, 8, 16], DT, tag="ksd")

        ps_proj_cm = tc.tile_pool(name="ps_proj", bufs=4, space="PSUM")
        ps_proj = ps_proj_cm.__enter__()

        def proj_block(p, csl, qsb, gsb, which):
            pps = ps_proj.tile([128, CHUNK], F32, tag="proj")
            if which == "q":
                # fp8 DoubleRow (see K projection note)
                for j in range(4):
                    nc.tensor.matmul(
                        pps, wsb["wq8"][:, 2 * j:2 * j + 2, ts(p, 128)],
                        X8[:, 2 * j:2 * j + 2, csl],
                        start=(j == 0), stop=(j == 3),
                        perf_mode=DR,
                    )
            else:
                for j in range(4):
                    nc.tensor.matmul(
                        pps, wsb["wg8"][:, 2 * j:2 * j + 2, ts(p, 128)],
                        X8[:, 2 * j:2 * j + 2, csl],
                        start=(j == 0), stop=(j == 3),
                        perf_mode=DR,
                    )
            if which == "q":
                r1 = pb_tmp.tile([128, CHUNK], F32, tag="br1")
                nc.scalar.activation(r1, pps, AF.Relu)
                m1 = pb_tmp.tile([128, CHUNK], F32, tag="bm1")
                nc.vector.tensor_scalar_min(m1, pps, 0.0)
                e1 = pb_tmp.tile([128, CHUNK], F32, tag="be1")
                nc.scalar.activation(e1, m1, AF.Exp)
                nc.gpsimd.tensor_add(qsb[:, p, :], r1, e1)
            else:
                nc.scalar.activation(gsb[:, p, :], pps, AF.Sigmoid,
                                     bias=bg_sb[:, p:p + 1])

        # ---- pass 1: Q and G projections for ALL chunks (collective-free
        # PE work that covers the AllReduce round-trip) ----
        qsbs, gsbs = [], []
        for ch in range(NCH):
            csl = ts(ch, CHUNK)
            qsb = pb_qg.tile([128, 8, CHUNK], DT, tag=f"qsb{ch}")
            gsb = pb_qg.tile([128, 8, CHUNK], DT, tag=f"gsb{ch}")
            qsbs.append(qsb)
            gsbs.append(gsb)
            for p in range(8):
                proj_block(p, csl, qsb, gsb, "q")
            if ch == 0:
                nc.sync.dma_start(out=kvf[:], in_=cc_out.ap()[:, :, :])
            for p in range(8):
                proj_block(p, csl, qsb, gsb, "g")

        ps_proj_cm.__exit__(None, None, None)

        # ---- collective prep + scheduling gate, ALL on the Pool engine.
        # No pass-1 matmul ever waits on Pool completions, so these
        # collective-dependent ops cannot poison pass-1 counting-semaphore
        # thresholds (they did when placed on ACT or DVE). ksd2/kvb2 gain a
        # zero-valued dependency on the LAST projection block's output:
        # the Tile scheduler's readiness model ignores AllReduce latency
        # and would otherwise hoist the first collective-dependent matmul
        # right behind chunk 0's projections, head-blocking the in-order
        # PE queue on the collective.
        nc.gpsimd.tensor_copy(kvb, kvf)
        nc.gpsimd.memset(ksd[:], 0.0)
        for p in range(8):
            nc.gpsimd.tensor_scalar_mul(
                ksd[0:64, p, 2 * p:2 * p + 1],
                kvf[0:64, p, 64:65], 1.0 / SCALE)
            nc.gpsimd.tensor_scalar_mul(
                ksd[64:128, p, 2 * p + 1:2 * p + 2],
                kvf[64:128, p, 64:65], 1.0 / SCALE)
        ksd2 = persist.tile([128, 8, 16], DT, tag="ksd2")
        kvb2 = persist.tile([128, 8, 65], DT, tag="kvb2")
        zl65 = pb_small.tile([128, 65], DT, tag="zl65")
        nc.gpsimd.tensor_scalar_mul(zl65, gsbs[NCH - 1][:, 7, 0:65], 0.0)
        for p in range(8):
            nc.gpsimd.tensor_add(ksd2[:, p, :], ksd[:, p, :],
                                 zl65[:, 0:16])
            nc.gpsimd.tensor_add(kvb2[:, p, :], kvb[:, p, :], zl65)

        ps_ops_cm = tc.tile_pool(name="ps_ops", bufs=2, space="PSUM")
        ps_z_cm = tc.tile_pool(name="ps_z", bufs=2, space="PSUM")
        ps_qk_cm = tc.tile_pool(name="ps_qk", bufs=2, space="PSUM")
        ps_y_cm = tc.tile_pool(name="ps_y", bufs=2, space="PSUM")
        ps_ops = ps_ops_cm.__enter__()
        ps_z = ps_z_cm.__enter__()
        ps_qk = ps_qk_cm.__enter__()
        ps_y = ps_y_cm.__enter__()

        # ---- pass 2: attention + output projection per chunk; each
        # chunk's qk runs one chunk ahead so the z reciprocal chain (DVE)
        # hides under the previous chunk's y matmuls. The whole pass is
        # deprioritized far past pass 1 so the scheduler's ready-heap
        # always prefers projection matmuls over collective-dependent ones.
        prio_cm = tc.high_priority(offset=-1000000)
        prio_cm.__enter__()

        def qk_mms(ch):
            qkps = ps_qk.tile([16, CHUNK], F32, tag="qk")
            for p in range(8):
                nc.tensor.matmul(
                    qkps, mm(ksd2[:, p, :]), mm(qsbs[ch][:, p, :]),
                    start=(p == 0), stop=(p == 7),
                    skip_group_check=True,
                )
            return qkps

        qkps_next = qk_mms(0)
        for ch in range(NCH):
            csl = ts(ch, CHUNK)
            qsb, gsb = qsbs[ch], gsbs[ch]
            qkps = qkps_next
            zq = pb_tmp.tile([16, CHUNK], F32, tag="zq")
            nc.vector.tensor_scalar_max(zq, qkps, CLAMP)
            zr = pb_tmp.tile([16, CHUNK], F32, tag="zr")
            nc.vector.reciprocal(zr, zq)
            zqr = pb_tmp.tile([16, CHUNK], DT, tag="zqr")
            nc.vector.tensor_copy(zqr, zr)

            asb = pb_big.tile([128, 8, CHUNK], DT, tag="asb")
            for p in range(8):
                ops_ = ps_ops.tile([128, CHUNK], F32, tag="ops")
                for rr in range(2):
                    pr = slice(64 * rr, 64 * rr + 64)
                    nc.tensor.matmul(
                        ops_[pr, :], mm(kvb2[pr, p, 0:64]),
                        mm(qsb[pr, p, :]),
                        start=True, stop=True,
                    )
                zbps = ps_z.tile([128, CHUNK], F32, tag="z")
                nc.tensor.matmul(zbps, mm(sel[:, p, :]), mm(zqr),
                                 start=True, stop=True)
                t1 = pb_tmp.tile([128, CHUNK], F32, tag="bt1")
                # each mul reads at most one PSUM operand (HW restriction)
                nc.vector.tensor_mul(t1, ops_, gsb[:, p, :])
                nc.vector.tensor_mul(asb[:, p, :], t1, zbps)
                if p == 1 and ch + 1 < NCH:
                    qkps_next = qk_mms(ch + 1)

            for d in range(8):
                yps = ps_y.tile([128, CHUNK], F32, tag="y")
                for fi in range(8):
                    nc.tensor.matmul(
                        yps, mm(wsb["wo"][:, fi, ts(d, 128)]),
                        mm(asb[:, fi, :]),
                        start=(fi == 0), stop=(fi == 7),
                    )
                ysb = pb_tmp.tile([128, CHUNK], DT, tag="ysb")
                nc.scalar.copy(ysb, yps)
                nc.sync.dma_start(out=y_d.ap()[ts(d, 128), csl],
                                  in_=ysb[:])

        prio_cm.__exit__(None, None, None)
        ps_y_cm.__exit__(None, None, None)
        ps_qk_cm.__exit__(None, None, None)
        ps_z_cm.__exit__(None, None, None)
        ps_ops_cm.__exit__(None, None, None)


def _np_dt(dt_mode):
    return ml_dtypes.bfloat16 if dt_mode == "bf16" else np.float32


def prep_inputs(x, Wq, Wk, Wv, Wg, bg, Wo, dt_mode=DT_MODE):
    npdt = _np_dt(dt_mode)
    f8 = ml_dtypes.float8_e4m3
    x_f = np.ascontiguousarray(np.asarray(x, np.float32).reshape(B * N, DIM))
    w_t = {}
    w_t["wv"] = np.ascontiguousarray(
        np.asarray(Wv, np.float32).T).astype(npdt)
    for nm, W in (("wq8", Wq), ("wk8", Wk), ("wg8", Wg)):
        w_t[nm] = np.ascontiguousarray(
            np.asarray(W, np.float32).T).astype(f8)
    w_t["wo"] = np.ascontiguousarray(
        np.asarray(Wo, np.float32).T).astype(npdt)
    bg_f = np.ascontiguousarray(np.asarray(bg, np.float32))
    in_maps = []
    for c in range(N_CORES):
        xt_T = np.ascontiguousarray(x_f[c * TPC:(c + 1) * TPC].T)
        m = {"xt": xt_T.astype(npdt), "x8": xt_T.astype(f8), "bg": bg_f}
        m.update(w_t)
        in_maps.append(m)
    return in_maps


def unshard_output(y_parts):
    out = np.empty((B * N, DIM), np.float32)
    for c in range(N_CORES):
        out[c * TPC:(c + 1) * TPC] = np.asarray(y_parts[c], np.float32).T
    return out.reshape(B, N, DIM)


def get_nc(dt_mode=DT_MODE):
    key = ("nc", dt_mode)
    if key not in _CACHE:
        _CACHE[key] = _build(dt_mode)
    return _CACHE[key]


def kernel(x, Wq, Wk, Wv, Wg, bg, Wo):
    from concourse.bass_utils import run_bass_kernel_spmd

    nc = get_nc()
    in_maps = prep_inputs(x, Wq, Wk, Wv, Wg, bg, Wo)
    res = run_bass_kernel_spmd(nc, in_maps, core_ids=list(range(N_CORES)))
    return unshard_output([res.results[c]["y"] for c in range(N_CORES)])


# revision 56
# speedup vs baseline: 1.1953x; 1.0653x over previous
"""Trainium2 Bass kernel for nn_GatedAttention (linear attention with sigmoid
gate).

Strategy: shard the 16384 token rows across 8 cores (2048 each; cores 2b,2b+1
hold batch b). Per core, two phases:
  A: K,V projections (token-major) + per-head kv' = K^T [V|1] accumulated in
     persistent PSUM tiles over all local tokens (the ones column folds k_sum
     into kv'). kv matmuls run one m-tile behind the projections so the elu
     chain never stalls the PE.
  -- pairwise AllReduce of kv' between the two cores sharing a batch --
  B: Q,G projections (feature-major), out^T = kv'^T @ Q per head, normalizer
     z = SCALE/max(q.k_sum,eps) applied via tiny selector matmuls, gate, and
     the final output projection, all feature-major.
DMAs are issued in consumer order (X/wk/wv before phase A, wq/wg/wo/bg/sel
after the first m-tile, collective output after ch0's Q matmuls) so counting
semaphores never serialize the PE behind unrelated transfers.
Host transposes x to feature-major and pre-transposes weights; output returns
feature-major bf16 per-core slabs that the host transposes back.
"""
import sys

sys.path.insert(0, "/opt/trn_rl_repo")

import numpy as np
import ml_dtypes

B, N, DIM = 4, 4096, 1024
HEADS, DH = 16, 64
SCALE = DH ** -0.5
N_CORES = 8
TPC = B * N // N_CORES      # 2048 tokens per core
NMT = TPC // 128            # 16 m-tiles (phase A)
CHUNK = 512
NCH = TPC // CHUNK          # 4 chunks (phase B)
CLAMP = 1e-6 / SCALE

DT_MODE = "bf16"            # "bf16" | "f32r" | "f32"

_CACHE = {}


def _build(dt_mode=DT_MODE, reps=1):
    import concourse.bacc as bacc
    import concourse.bass as bass
    import concourse.tile as tile
    from concourse import mybir

    AF = mybir.ActivationFunctionType
    F32 = mybir.dt.float32
    F8 = mybir.dt.float8e4
    DR = mybir.MatmulPerfMode.DoubleRow
    DT = mybir.dt.bfloat16 if dt_mode == "bf16" else mybir.dt.float32

    def mm(ap):
        return ap.bitcast(mybir.dt.float32r) if dt_mode == "f32r" else ap

    ts = bass.ts

    nc = bacc.Bacc("TRN2", target_bir_lowering=False, debug=False,
                   num_devices=N_CORES)
    xt = nc.dram_tensor("xt", [DIM, TPC], DT, kind="ExternalInput")
    x8_d = nc.dram_tensor("x8", [DIM, TPC], F8, kind="ExternalInput")
    w_in = {}
    for nm in ("wv", "wo"):
        w_in[nm] = nc.dram_tensor(nm, [DIM, DIM], DT, kind="ExternalInput")
    for nm in ("wk8", "wq8", "wg8"):
        w_in[nm] = nc.dram_tensor(nm, [DIM, DIM], F8, kind="ExternalInput")
    bg_d = nc.dram_tensor("bg", [DIM], F32, kind="ExternalInput")
    y_d = nc.dram_tensor("y", [DIM, TPC], DT, kind="ExternalOutput")
    cc_in = nc.dram_tensor("cc_in", [128, 8, 65], F32)
    cc_out = nc.dram_tensor("cc_out", [128, 8, 65], F32)

    with tile.TileContext(nc, num_cores=N_CORES) as tc:
        with (
            tc.tile_pool(name="persist", bufs=1) as persist,
            tc.tile_pool(name="pb_big", bufs=2) as pb_big,
        ):
            X = persist.tile([128, 8, TPC], DT, tag="x")
            X8 = persist.tile([128, 8, TPC], F8, tag="x8")
            wsb = {}
            wsb["wo"] = persist.tile([128, 8, DIM], DT, tag="wo", name="wo")
            for nm in ("wq8", "wg8"):
                wsb[nm] = persist.tile([128, 8, DIM], F8, tag=nm, name=nm)
            bg_sb = persist.tile([128, 8], F32, tag="bg")
            sel_np = np.zeros((16, 8, 128), _np_dt(dt_mode))
            for p in range(8):
                sel_np[2 * p, p, 0:64] = 1.0
                sel_np[2 * p + 1, p, 64:128] = 1.0
            sel_d = nc.inline_tensor(sel_np, name="sel_const")
            sel = persist.tile([16, 8, 128], DT, tag="sel")

            for _rep in range(reps):
                _phases(nc, tc, bass, mybir, AF, F32, DT, mm, ts, X, wsb,
                        bg_sb, sel, sel_d, w_in, xt, bg_d, cc_in, cc_out, y_d,
                        tc_pools=(persist, pb_big), X8=X8, x8_d=x8_d,
                        F8=F8, DR=DR)
    nc.compile()
    return nc


def _phases(nc, tc, bass, mybir, AF, F32, DT, mm, ts, X, wsb, bg_sb, sel,
            sel_d, w_in, xt, bg_d, cc_in, cc_out, y_d, tc_pools, X8, x8_d,
            F8, DR):
    persist, pb_big = tc_pools
    # ---------------- phase A ----------------
    with (
        tc.tile_pool(name="pa_w", bufs=1) as pa_w,
        tc.tile_pool(name="pa_tmp", bufs=2) as pa_tmp,
        tc.tile_pool(name="pa_ps", bufs=2, space="PSUM") as pa_ps,
        tc.tile_pool(name="kv_ps", bufs=1, space="PSUM") as kv_pool,
    ):
        wsb["wk8"] = pa_w.tile([128, 8, DIM], F8, tag="wk8", name="wk8")
        wsb["wv"] = pa_w.tile([128, 8, DIM], DT, tag="wv", name="wv")
        # consumer-ordered DMA issue: phase A inputs first, interleaved by
        # contraction chunk so the first m-tile can start ASAP (X/wv are
        # only needed once the first V-projection starts, ~1.7us after K).
        for i in range(8):
            nc.sync.dma_start(out=X8[:, i, :], in_=x8_d.ap()[ts(i, 128), :])
            nc.sync.dma_start(out=wsb["wk8"][:, i, :],
                              in_=w_in["wk8"].ap()[ts(i, 128), :])
        for i in range(8):
            nc.sync.dma_start(out=X[:, i, :], in_=xt.ap()[ts(i, 128), :])
            nc.sync.dma_start(out=wsb["wv"][:, i, :],
                              in_=w_in["wv"].ap()[ts(i, 128), :])

        # HAM warm-up: the PE clock sits at 1.2GHz until ~3.4us of sustained
        # matmul activity, and the input DMAs take ~12us to land AND pace
        # the first two m-tiles. Dummy matmuls on zeroed scratch fill the
        # initial window, and more are interleaved into the DMA-paced
        # m-tiles (below) so the activity window never sees an idle gap.
        warm_cm = tc.tile_pool(name="warm", bufs=1)
        warm_pool = warm_cm.__enter__()
        warm_ps_cm = tc.tile_pool(name="warm_ps", bufs=1, space="PSUM")
        warm_ps = warm_ps_cm.__enter__()
        wa = warm_pool.tile([128, 128], DT, tag="wa")
        wb = warm_pool.tile([128, 512], DT, tag="wb")
        nc.vector.memset(wa[:], 0.0)
        nc.vector.memset(wb[:], 0.0)
        wp = warm_ps.tile([128, 512], F32, tag="wp")
        for _ in range(14):
            nc.tensor.matmul(wp, mm(wa), mm(wb), start=True, stop=True)

        # persistent PSUM accumulators for kv': 2 tiles x [128, 4, 128]
        # (tile w holds heads 8w..8w+7: slice j rows 0:64 = head 8w+2j,
        #  rows 64:128 = head 8w+2j+1; j-slices padded to 128 floats so a
        #  matmul output never crosses a PSUM bank boundary)
        kv_ps = [kv_pool.tile([128, 4, 128], F32, tag=f"kv{w}",
                              name=f"kv{w}") for w in range(2)]
        ksb_hist = [None, None]
        vp_hist = [None, None]

        def kv_mms(mt):
            ksb_o = ksb_hist[mt % 2]
            vp_o = vp_hist[mt % 2]
            for w in range(2):
                for j in range(4):
                    for c in range(2):
                        h = 8 * w + 2 * j + c
                        # start only on the FIRST matmul touching this bank's
                        # partition plane: start_tensor_calc marks the whole
                        # 2KB zero-region pending, so a second start=True in
                        # the same bank would re-poison already-written
                        # slices and turn later accumulates into overwrites.
                        nc.tensor.matmul(
                            kv_ps[w][64 * c:64 * c + 64, j, 0:65],
                            mm(ksb_o[:, ts(h, 64)]),
                            mm(vp_o[:, h, :]),
                            start=(mt == 0 and j == 0),
                            stop=(mt == NMT - 1),
                            skip_group_check=True,
                        )

        for mt in range(NMT):
            msl = ts(mt, 128)
            kps = pa_ps.tile([128, 1024], F32, tag="proj")
            # K projection in fp8 DoubleRow: contraction pairs of 128-chunks
            # (effective K=256 per matmul, ~1.4-2x PE throughput); fp8 on
            # Q/K is accuracy-safe because the normalizer z cancels most of
            # the quantization error (measured rel 0.0069 vs 0.02 budget).
            for j in range(4):
                for o in range(2):
                    nc.tensor.matmul(
                        kps[:, ts(o, 512)],
                        X8[:, 2 * j:2 * j + 2, msl],
                        wsb["wk8"][:, 2 * j:2 * j + 2, ts(o, 512)],
                        start=(j == 0), stop=(j == 3),
                        perf_mode=DR,
                    )
                if mt < 2:
                    nc.tensor.matmul(wp, mm(wa), mm(wb), start=True,
                                     stop=True)
            if mt == 1:
                # phase-B inputs: issued after the first m-tile's matmuls so
                # phase-A waits never count these transfers.
                bg_ap = bg_d.ap()
                nc.sync.dma_start(
                    out=bg_sb[:],
                    in_=bass.AP(tensor=bg_ap.tensor, offset=0,
                                ap=[[1, 128], [128, 8]]),
                )
                nc.sync.dma_start(out=sel[:], in_=sel_d.ap())
                for i in range(8):
                    for nm in ("wq8", "wg8", "wo"):
                        nc.sync.dma_start(out=wsb[nm][:, i, :],
                                          in_=w_in[nm].ap()[ts(i, 128), :])
            r1 = pa_tmp.tile([128, 1024], F32, tag="r1")
            nc.scalar.activation(r1, kps, AF.Relu)
            m1 = pa_tmp.tile([128, 1024], F32, tag="m1")
            nc.vector.tensor_scalar_min(m1, kps, 0.0)
            e1 = pa_tmp.tile([128, 1024], F32, tag="e1")
            nc.scalar.activation(e1, m1, AF.Exp)
            ksb = pa_tmp.tile([128, 1024], DT, tag="ksb")
            nc.gpsimd.tensor_add(ksb, r1, e1)
            ksb_hist[mt % 2] = ksb

            vps = pa_ps.tile([128, 16, 64], F32, tag="proj")
            for i in range(8):
                for o in range(2):
                    nc.tensor.matmul(
                        vps[:, ts(o, 8), :],
                        mm(X[:, i, msl]),
                        mm(wsb["wv"][:, i, ts(o, 512)]),
                        start=(i == 0), stop=(i == 7),
                    )
                if mt < 2:
                    nc.tensor.matmul(wp, mm(wa), mm(wb), start=True,
                                     stop=True)
            vp = pa_tmp.tile([128, 16, 65], DT, tag="vp")
            nc.vector.memset(vp[:, :, 64:65], 1.0)
            nc.scalar.copy(vp[:, :, 0:64], vps[:, :, :])
            vp_hist[mt % 2] = vp

            if mt > 0:
                kv_mms(mt - 1)
        kv_mms(NMT - 1)
        warm_ps_cm.__exit__(None, None, None)
        warm_cm.__exit__(None, None, None)

        kv_sb = pa_tmp.tile([128, 8, 65], F32, tag="kv_sb", bufs=1,
                            name="kv_sb")
        for w in range(2):
            nc.vector.tensor_copy(kv_sb[:, 4 * w:4 * w + 4, :],
                                  kv_ps[w][:, :, 0:65])
        nc.sync.dma_start(out=cc_in.ap()[:, :, :], in_=kv_sb[:])

    nc.gpsimd.collective_compute(
        "AllReduce",
        mybir.AluOpType.add,
        replica_groups=[[0, 1], [2, 3], [4, 5], [6, 7]],
        ins=[cc_in.ap().opt()],
        outs=[cc_out.ap().opt()],
    )

    # ---------------- phase B ----------------
    with (
        tc.tile_pool(name="pb_tmp", bufs=2) as pb_tmp,
        tc.tile_pool(name="pb_small", bufs=1) as pb_small,
        tc.tile_pool(name="pb_qg", bufs=1) as pb_qg,
    ):
        # collective results live in the persist pool: fresh SBUF, so the
        # kvf DMA has no write-after-read wait on phase-A consumers.
        kvf = persist.tile([128, 8, 65], F32, tag="kvf")
        kvb = persist.tile([128, 8, 65], DT, tag="kvb")
        ksd = persist.tile([128, 8, 16], DT, tag="ksd")

        ps_proj_cm = tc.tile_pool(name="ps_proj", bufs=6, space="PSUM")
        ps_proj = ps_proj_cm.__enter__()

        def proj_block(p, csl, qsb, gsb, which):
            pps = ps_proj.tile([128, CHUNK], F32, tag="proj")
            wname = "wq8" if which == "q" else "wg8"
            # fp8 DoubleRow (see K projection note; G measured rel 0.0127
            # alone, 0.0162 combined with Q/K — inside the 0.02 budget)
            for j in range(4):
                nc.tensor.matmul(
                    pps, wsb[wname][:, 2 * j:2 * j + 2, ts(p, 128)],
                    X8[:, 2 * j:2 * j + 2, csl],
                    start=(j == 0), stop=(j == 3),
                    perf_mode=DR,
                )
            if which == "q":
                # relu on DVE, not ACT: with fp8 projections the PE pace is
                # ~0.85us/block and an ACT-side relu+exp chain (1.2us) makes
                # pass 1 ACT-bound (PSUM-reuse stalls + HAM oscillation).
                r1 = pb_tmp.tile([128, CHUNK], F32, tag="br1")
                nc.vector.tensor_scalar_max(r1, pps, 0.0)
                m1 = pb_tmp.tile([128, CHUNK], F32, tag="bm1")
                nc.vector.tensor_scalar_min(m1, pps, 0.0)
                e1 = pb_tmp.tile([128, CHUNK], F32, tag="be1")
                nc.scalar.activation(e1, m1, AF.Exp)
                nc.gpsimd.tensor_add(qsb[:, p, :], r1, e1)
            else:
                nc.scalar.activation(gsb[:, p, :], pps, AF.Sigmoid,
                                     bias=bg_sb[:, p:p + 1])

        # ---- pass 1: Q and G projections for ALL chunks (collective-free
        # PE work that covers the AllReduce round-trip) ----
        qsbs, gsbs = [], []
        for ch in range(NCH):
            csl = ts(ch, CHUNK)
            qsb = pb_qg.tile([128, 8, CHUNK], DT, tag=f"qsb{ch}")
            gsb = pb_qg.tile([128, 8, CHUNK], DT, tag=f"gsb{ch}")
            qsbs.append(qsb)
            gsbs.append(gsb)
            for p in range(8):
                proj_block(p, csl, qsb, gsb, "q")
            if ch == 0:
                nc.sync.dma_start(out=kvf[:], in_=cc_out.ap()[:, :, :])
            for p in range(8):
                proj_block(p, csl, qsb, gsb, "g")

        ps_proj_cm.__exit__(None, None, None)

        # ---- collective prep + scheduling gate, ALL on the Pool engine.
        # No pass-1 matmul ever waits on Pool completions, so these
        # collective-dependent ops cannot poison pass-1 counting-semaphore
        # thresholds (they did when placed on ACT or DVE). ksd2/kvb2 gain a
        # zero-valued dependency on the LAST projection block's output:
        # the Tile scheduler's readiness model ignores AllReduce latency
        # and would otherwise hoist the first collective-dependent matmul
        # right behind chunk 0's projections, head-blocking the in-order
        # PE queue on the collective.
        nc.gpsimd.tensor_copy(kvb, kvf)
        nc.gpsimd.memset(ksd[:], 0.0)
        for p in range(8):
            nc.gpsimd.tensor_scalar_mul(
                ksd[0:64, p, 2 * p:2 * p + 1],
                kvf[0:64, p, 64:65], 1.0 / SCALE)
            nc.gpsimd.tensor_scalar_mul(
                ksd[64:128, p, 2 * p + 1:2 * p + 2],
                kvf[64:128, p, 64:65], 1.0 / SCALE)
        ksd2 = persist.tile([128, 8, 16], DT, tag="ksd2")
        kvb2 = persist.tile([128, 8, 65], DT, tag="kvb2")
        zl65 = pb_small.tile([128, 65], DT, tag="zl65")
        # gate on p6, not p7: frees the gate chain to overlap the final
        # projection block (still ~107us of guaranteed collective coverage)
        nc.gpsimd.tensor_scalar_mul(zl65, gsbs[NCH - 1][:, 6, 0:65], 0.0)
        for p in range(8):
            nc.gpsimd.tensor_add(ksd2[:, p, :], ksd[:, p, :],
                                 zl65[:, 0:16])
            nc.gpsimd.tensor_add(kvb2[:, p, :], kvb[:, p, :], zl65)

        ps_ops_cm = tc.tile_pool(name="ps_ops", bufs=2, space="PSUM")
        ps_z_cm = tc.tile_pool(name="ps_z", bufs=2, space="PSUM")
        ps_qk_cm = tc.tile_pool(name="ps_qk", bufs=2, space="PSUM")
        ps_y_cm = tc.tile_pool(name="ps_y", bufs=2, space="PSUM")
        ps_ops = ps_ops_cm.__enter__()
        ps_z = ps_z_cm.__enter__()
        ps_qk = ps_qk_cm.__enter__()
        ps_y = ps_y_cm.__enter__()

        # ---- pass 2: attention + output projection per chunk; each
        # chunk's qk runs one chunk ahead so the z reciprocal chain (DVE)
        # hides under the previous chunk's y matmuls. The whole pass is
        # deprioritized far past pass 1 so the scheduler's ready-heap
        # always prefers projection matmuls over collective-dependent ones.
        prio_cm = tc.high_priority(offset=-1000000)
        prio_cm.__enter__()

        def qk_mms(ch):
            qkps = ps_qk.tile([16, CHUNK], F32, tag="qk")
            for p in range(8):
                nc.tensor.matmul(
                    qkps, mm(ksd2[:, p, :]), mm(qsbs[ch][:, p, :]),
                    start=(p == 0), stop=(p == 7),
                    skip_group_check=True,
                )
            return qkps

        qkps_next = qk_mms(0)
        for ch in range(NCH):
            csl = ts(ch, CHUNK)
            qsb, gsb = qsbs[ch], gsbs[ch]
            qkps = qkps_next
            zq = pb_tmp.tile([16, CHUNK], F32, tag="zq")
            nc.vector.tensor_scalar_max(zq, qkps, CLAMP)
            zr = pb_tmp.tile([16, CHUNK], F32, tag="zr")
            nc.vector.reciprocal(zr, zq)
            zqr = pb_tmp.tile([16, CHUNK], DT, tag="zqr")
            nc.vector.tensor_copy(zqr, zr)

            asb = pb_big.tile([128, 8, CHUNK], DT, tag="asb")
            for p in range(8):
                ops_ = ps_ops.tile([128, CHUNK], F32, tag="ops")
                for rr in range(2):
                    pr = slice(64 * rr, 64 * rr + 64)
                    nc.tensor.matmul(
                        ops_[pr, :], mm(kvb2[pr, p, 0:64]),
                        mm(qsb[pr, p, :]),
                        start=True, stop=True,
                    )
                zbps = ps_z.tile([128, CHUNK], F32, tag="z")
                nc.tensor.matmul(zbps, mm(sel[:, p, :]), mm(zqr),
                                 start=True, stop=True)
                t1 = pb_tmp.tile([128, CHUNK], F32, tag="bt1")
                # each mul reads at most one PSUM operand (HW restriction)
                nc.vector.tensor_mul(t1, ops_, gsb[:, p, :])
                nc.vector.tensor_mul(asb[:, p, :], t1, zbps)
                if p == 1 and ch + 1 < NCH:
                    qkps_next = qk_mms(ch + 1)

            for d in range(8):
                yps = ps_y.tile([128, CHUNK], F32, tag="y")
                for fi in range(8):
                    nc.tensor.matmul(
                        yps, mm(wsb["wo"][:, fi, ts(d, 128)]),
                        mm(asb[:, fi, :]),
                        start=(fi == 0), stop=(fi == 7),
                    )
                ysb = pb_tmp.tile([128, CHUNK], DT, tag="ysb")
                nc.scalar.copy(ysb, yps)
                nc.sync.dma_start(out=y_d.ap()[ts(d, 128), csl],
                                  in_=ysb[:])

        prio_cm.__exit__(None, None, None)
        ps_y_cm.__exit__(None, None, None)
        ps_qk_cm.__exit__(None, None, None)
        ps_z_cm.__exit__(None, None, None)
        ps_ops_cm.__exit__(None, None, None)


def _np_dt(dt_mode):
    return ml_dtypes.bfloat16 if dt_mode == "bf16" else np.float32


def prep_inputs(x, Wq, Wk, Wv, Wg, bg, Wo, dt_mode=DT_MODE):
    npdt = _np_dt(dt_mode)
    f8 = ml_dtypes.float8_e4m3
    x_f = np.ascontiguousarray(np.asarray(x, np.float32).reshape(B * N, DIM))
    w_t = {}
    w_t["wv"] = np.ascontiguousarray(
        np.asarray(Wv, np.float32).T).astype(npdt)
    for nm, W in (("wq8", Wq), ("wk8", Wk), ("wg8", Wg)):
        w_t[nm] = np.ascontiguousarray(
            np.asarray(W, np.float32).T).astype(f8)
    w_t["wo"] = np.ascontiguousarray(
        np.asarray(Wo, np.float32).T).astype(npdt)
    bg_f = np.ascontiguousarray(np.asarray(bg, np.float32))
    in_maps = []
    for c in range(N_CORES):
        xt_T = np.ascontiguousarray(x_f[c * TPC:(c + 1) * TPC].T)
        m = {"xt": xt_T.astype(npdt), "x8": xt_T.astype(f8), "bg": bg_f}
        m.update(w_t)
        in_maps.append(m)
    return in_maps


def unshard_output(y_parts):
    out = np.empty((B * N, DIM), np.float32)
    for c in range(N_CORES):
        out[c * TPC:(c + 1) * TPC] = np.asarray(y_parts[c], np.float32).T
    return out.reshape(B, N, DIM)


def get_nc(dt_mode=DT_MODE):
    key = ("nc", dt_mode)
    if key not in _CACHE:
        _CACHE[key] = _build(dt_mode)
    return _CACHE[key]


def kernel(x, Wq, Wk, Wv, Wg, bg, Wo):
    from concourse.bass_utils import run_bass_kernel_spmd

    nc = get_nc()
    in_maps = prep_inputs(x, Wq, Wk, Wv, Wg, bg, Wo)
    res = run_bass_kernel_spmd(nc, in_maps, core_ids=list(range(N_CORES)))
    return unshard_output([res.results[c]["y"] for c in range(N_CORES)])


# revision 61
# speedup vs baseline: 1.2612x; 1.0552x over previous
"""Trainium2 Bass kernel for nn_GatedAttention (linear attention with sigmoid
gate).

Strategy: shard the 16384 token rows across 8 cores (2048 each; cores 2b,2b+1
hold batch b). Per core, two phases:
  A: K,V projections (token-major) + per-head kv' = K^T [V|1] accumulated in
     persistent PSUM tiles over all local tokens (the ones column folds k_sum
     into kv'). kv matmuls run one m-tile behind the projections so the elu
     chain never stalls the PE.
  -- pairwise AllReduce of kv' between the two cores sharing a batch --
  B: Q,G projections (feature-major), out^T = kv'^T @ Q per head, normalizer
     z = SCALE/max(q.k_sum,eps) applied via tiny selector matmuls, gate, and
     the final output projection, all feature-major.
DMAs are issued in consumer order (X/wk/wv before phase A, wq/wg/wo/bg/sel
after the first m-tile, collective output after ch0's Q matmuls) so counting
semaphores never serialize the PE behind unrelated transfers.
Host transposes x to feature-major and pre-transposes weights; output returns
feature-major bf16 per-core slabs that the host transposes back.
"""
import sys

sys.path.insert(0, "/opt/trn_rl_repo")

import numpy as np
import ml_dtypes

B, N, DIM = 4, 4096, 1024
HEADS, DH = 16, 64
SCALE = DH ** -0.5
N_CORES = 8
TPC = B * N // N_CORES      # 2048 tokens per core
NMT = TPC // 128            # 16 m-tiles (phase A)
CHUNK = 512
NCH = TPC // CHUNK          # 4 chunks (phase B)
CLAMP = 1e-6  # ksd carries plain k_sum; SCALE applied in the zqr multiply

DT_MODE = "bf16"            # "bf16" | "f32r" | "f32"

_CACHE = {}


def _build(dt_mode=DT_MODE, reps=1):
    import concourse.bacc as bacc
    import concourse.bass as bass
    import concourse.tile as tile
    from concourse import mybir

    AF = mybir.ActivationFunctionType
    F32 = mybir.dt.float32
    F8 = mybir.dt.float8e4
    DR = mybir.MatmulPerfMode.DoubleRow
    DT = mybir.dt.bfloat16 if dt_mode == "bf16" else mybir.dt.float32

    def mm(ap):
        return ap.bitcast(mybir.dt.float32r) if dt_mode == "f32r" else ap

    ts = bass.ts

    nc = bacc.Bacc("TRN2", target_bir_lowering=False, debug=False,
                   num_devices=N_CORES)
    xt = nc.dram_tensor("xt", [DIM, TPC], DT, kind="ExternalInput")
    x8_d = nc.dram_tensor("x8", [DIM, TPC], F8, kind="ExternalInput")
    w_in = {}
    for nm in ("wv", "wo"):
        w_in[nm] = nc.dram_tensor(nm, [DIM, DIM], DT, kind="ExternalInput")
    for nm in ("wk8", "wq8", "wg8"):
        w_in[nm] = nc.dram_tensor(nm, [DIM, DIM], F8, kind="ExternalInput")
    bg_d = nc.dram_tensor("bg", [DIM], F32, kind="ExternalInput")
    y_d = nc.dram_tensor("y", [DIM, TPC], DT, kind="ExternalOutput")
    cc_in = nc.dram_tensor("cc_in", [128, 8, 65], F32)
    cc_out = nc.dram_tensor("cc_out", [128, 8, 65], F32)

    with tile.TileContext(nc, num_cores=N_CORES) as tc:
        with (
            tc.tile_pool(name="persist", bufs=1) as persist,
            tc.tile_pool(name="pb_big", bufs=2) as pb_big,
        ):
            X = persist.tile([128, 8, TPC], DT, tag="x")
            X8 = persist.tile([128, 8, TPC], F8, tag="x8")
            wsb = {}
            wsb["wo"] = persist.tile([128, 8, DIM], DT, tag="wo", name="wo")
            for nm in ("wq8", "wg8"):
                wsb[nm] = persist.tile([128, 8, DIM], F8, tag=nm, name=nm)
            bg_sb = persist.tile([128, 8], F32, tag="bg")
            sel_np = np.zeros((16, 8, 128), _np_dt(dt_mode))
            for p in range(8):
                sel_np[2 * p, p, 0:64] = 1.0
                sel_np[2 * p + 1, p, 64:128] = 1.0
            sel_d = nc.inline_tensor(sel_np, name="sel_const")
            sel = persist.tile([16, 8, 128], DT, tag="sel")

            for _rep in range(reps):
                _phases(nc, tc, bass, mybir, AF, F32, DT, mm, ts, X, wsb,
                        bg_sb, sel, sel_d, w_in, xt, bg_d, cc_in, cc_out, y_d,
                        tc_pools=(persist, pb_big), X8=X8, x8_d=x8_d,
                        F8=F8, DR=DR)
    nc.compile()
    return nc


def _phases(nc, tc, bass, mybir, AF, F32, DT, mm, ts, X, wsb, bg_sb, sel,
            sel_d, w_in, xt, bg_d, cc_in, cc_out, y_d, tc_pools, X8, x8_d,
            F8, DR):
    persist, pb_big = tc_pools
    # ---------------- phase A ----------------
    with (
        tc.tile_pool(name="pa_w", bufs=1) as pa_w,
        tc.tile_pool(name="pa_tmp", bufs=2) as pa_tmp,
        tc.tile_pool(name="pa_ps", bufs=2, space="PSUM") as pa_ps,
        tc.tile_pool(name="kv_ps", bufs=1, space="PSUM") as kv_pool,
    ):
        wsb["wk8"] = pa_w.tile([128, 8, DIM], F8, tag="wk8", name="wk8")
        wsb["wv"] = pa_w.tile([128, 8, DIM], DT, tag="wv", name="wv")
        # consumer-ordered DMA issue: phase A inputs first, interleaved by
        # contraction chunk so the first m-tile can start ASAP (X/wv are
        # only needed once the first V-projection starts, ~1.7us after K).
        for i in range(8):
            nc.sync.dma_start(out=X8[:, i, :], in_=x8_d.ap()[ts(i, 128), :])
            nc.sync.dma_start(out=wsb["wk8"][:, i, :],
                              in_=w_in["wk8"].ap()[ts(i, 128), :])
        for i in range(8):
            nc.sync.dma_start(out=X[:, i, :], in_=xt.ap()[ts(i, 128), :])
            nc.sync.dma_start(out=wsb["wv"][:, i, :],
                              in_=w_in["wv"].ap()[ts(i, 128), :])

        # HAM warm-up: the PE clock sits at 1.2GHz until ~3.4us of sustained
        # matmul activity, and the input DMAs take ~12us to land AND pace
        # the first two m-tiles. Dummy matmuls on zeroed scratch fill the
        # initial window, and more are interleaved into the DMA-paced
        # m-tiles (below) so the activity window never sees an idle gap.
        warm_cm = tc.tile_pool(name="warm", bufs=1)
        warm_pool = warm_cm.__enter__()
        warm_ps_cm = tc.tile_pool(name="warm_ps", bufs=1, space="PSUM")
        warm_ps = warm_ps_cm.__enter__()
        wa = warm_pool.tile([128, 128], DT, tag="wa")
        wb = warm_pool.tile([128, 512], DT, tag="wb")
        nc.vector.memset(wa[:], 0.0)
        nc.vector.memset(wb[:], 0.0)
        wp = warm_ps.tile([128, 512], F32, tag="wp")
        for _ in range(14):
            nc.tensor.matmul(wp, mm(wa), mm(wb), start=True, stop=True)

        # persistent PSUM accumulators for kv': 2 tiles x [128, 4, 128]
        # (tile w holds heads 8w..8w+7: slice j rows 0:64 = head 8w+2j,
        #  rows 64:128 = head 8w+2j+1; j-slices padded to 128 floats so a
        #  matmul output never crosses a PSUM bank boundary)
        kv_ps = [kv_pool.tile([128, 4, 128], F32, tag=f"kv{w}",
                              name=f"kv{w}") for w in range(2)]
        ksb_hist = [None, None]
        vp_hist = [None, None]

        def kv_mms(mt):
            ksb_o = ksb_hist[mt % 2]
            vp_o = vp_hist[mt % 2]
            for w in range(2):
                for j in range(4):
                    for c in range(2):
                        h = 8 * w + 2 * j + c
                        # start only on the FIRST matmul touching this bank's
                        # partition plane: start_tensor_calc marks the whole
                        # 2KB zero-region pending, so a second start=True in
                        # the same bank would re-poison already-written
                        # slices and turn later accumulates into overwrites.
                        nc.tensor.matmul(
                            kv_ps[w][64 * c:64 * c + 64, j, 0:65],
                            mm(ksb_o[:, ts(h, 64)]),
                            mm(vp_o[:, h, :]),
                            start=(mt == 0 and j == 0),
                            stop=(mt == NMT - 1),
                            skip_group_check=True,
                        )

        for mt in range(NMT):
            msl = ts(mt, 128)
            kps = pa_ps.tile([128, 1024], F32, tag="proj")
            # K projection in fp8 DoubleRow: contraction pairs of 128-chunks
            # (effective K=256 per matmul, ~1.4-2x PE throughput); fp8 on
            # Q/K is accuracy-safe because the normalizer z cancels most of
            # the quantization error (measured rel 0.0069 vs 0.02 budget).
            for j in range(4):
                for o in range(2):
                    nc.tensor.matmul(
                        kps[:, ts(o, 512)],
                        X8[:, 2 * j:2 * j + 2, msl],
                        wsb["wk8"][:, 2 * j:2 * j + 2, ts(o, 512)],
                        start=(j == 0), stop=(j == 3),
                        perf_mode=DR,
                    )
                if mt < 2:
                    nc.tensor.matmul(wp, mm(wa), mm(wb), start=True,
                                     stop=True)
            if mt == 1:
                # phase-B inputs: issued after the first m-tile's matmuls so
                # phase-A waits never count these transfers.
                bg_ap = bg_d.ap()
                nc.sync.dma_start(
                    out=bg_sb[:],
                    in_=bass.AP(tensor=bg_ap.tensor, offset=0,
                                ap=[[1, 128], [128, 8]]),
                )
                nc.sync.dma_start(out=sel[:], in_=sel_d.ap())
                for i in range(8):
                    for nm in ("wq8", "wg8", "wo"):
                        nc.sync.dma_start(out=wsb[nm][:, i, :],
                                          in_=w_in[nm].ap()[ts(i, 128), :])
            r1 = pa_tmp.tile([128, 1024], F32, tag="r1")
            nc.scalar.activation(r1, kps, AF.Relu)
            m1 = pa_tmp.tile([128, 1024], F32, tag="m1")
            nc.vector.tensor_scalar_min(m1, kps, 0.0)
            e1 = pa_tmp.tile([128, 1024], F32, tag="e1")
            nc.scalar.activation(e1, m1, AF.Exp)
            ksb = pa_tmp.tile([128, 1024], DT, tag="ksb")
            nc.gpsimd.tensor_add(ksb, r1, e1)
            ksb_hist[mt % 2] = ksb

            vps = pa_ps.tile([128, 16, 64], F32, tag="proj")
            for i in range(8):
                for o in range(2):
                    nc.tensor.matmul(
                        vps[:, ts(o, 8), :],
                        mm(X[:, i, msl]),
                        mm(wsb["wv"][:, i, ts(o, 512)]),
                        start=(i == 0), stop=(i == 7),
                    )
                if mt < 2:
                    nc.tensor.matmul(wp, mm(wa), mm(wb), start=True,
                                     stop=True)
            vp = pa_tmp.tile([128, 16, 65], DT, tag="vp")
            nc.vector.memset(vp[:, :, 64:65], 1.0)
            nc.scalar.copy(vp[:, :, 0:64], vps[:, :, :])
            vp_hist[mt % 2] = vp

            if mt > 0:
                kv_mms(mt - 1)
        kv_mms(NMT - 1)
        warm_ps_cm.__exit__(None, None, None)
        warm_cm.__exit__(None, None, None)

        kv_sb = pa_tmp.tile([128, 8, 65], F32, tag="kv_sb", bufs=1,
                            name="kv_sb")
        for w in range(2):
            nc.vector.tensor_copy(kv_sb[:, 4 * w:4 * w + 4, :],
                                  kv_ps[w][:, :, 0:65])
        nc.sync.dma_start(out=cc_in.ap()[:, :, :], in_=kv_sb[:])

    nc.gpsimd.collective_compute(
        "AllReduce",
        mybir.AluOpType.add,
        replica_groups=[[0, 1], [2, 3], [4, 5], [6, 7]],
        ins=[cc_in.ap().opt()],
        outs=[cc_out.ap().opt()],
    )

    # ---------------- phase B ----------------
    with (
        tc.tile_pool(name="pb_tmp", bufs=2) as pb_tmp,
        tc.tile_pool(name="pb_small", bufs=1) as pb_small,
        tc.tile_pool(name="pb_qg", bufs=1) as pb_qg,
    ):
        # collective results live in the persist pool: fresh SBUF, so the
        # kvf DMA has no write-after-read wait on phase-A consumers.
        kvf = persist.tile([128, 8, 65], F32, tag="kvf")
        kvb = persist.tile([128, 8, 65], DT, tag="kvb")
        ksd = persist.tile([128, 8, 16], DT, tag="ksd")

        ps_proj_cm = tc.tile_pool(name="ps_proj", bufs=6, space="PSUM")
        ps_proj = ps_proj_cm.__enter__()

        def proj_block(p, csl, qsb, gsb, which):
            pps = ps_proj.tile([128, CHUNK], F32, tag="proj")
            wname = "wq8" if which == "q" else "wg8"
            # fp8 DoubleRow (see K projection note; G measured rel 0.0127
            # alone, 0.0162 combined with Q/K — inside the 0.02 budget)
            for j in range(4):
                nc.tensor.matmul(
                    pps, wsb[wname][:, 2 * j:2 * j + 2, ts(p, 128)],
                    X8[:, 2 * j:2 * j + 2, csl],
                    start=(j == 0), stop=(j == 3),
                    perf_mode=DR,
                )
            if which == "q":
                # relu on DVE, not ACT: with fp8 projections the PE pace is
                # ~0.85us/block and an ACT-side relu+exp chain (1.2us) makes
                # pass 1 ACT-bound (PSUM-reuse stalls + HAM oscillation).
                r1 = pb_tmp.tile([128, CHUNK], F32, tag="br1")
                nc.vector.tensor_scalar_max(r1, pps, 0.0)
                m1 = pb_tmp.tile([128, CHUNK], F32, tag="bm1")
                nc.vector.tensor_scalar_min(m1, pps, 0.0)
                e1 = pb_tmp.tile([128, CHUNK], F32, tag="be1")
                nc.scalar.activation(e1, m1, AF.Exp)
                nc.gpsimd.tensor_add(qsb[:, p, :], r1, e1)
            else:
                nc.scalar.activation(gsb[:, p, :], pps, AF.Sigmoid,
                                     bias=bg_sb[:, p:p + 1])

        # ---- pass 1: Q and G projections for ALL chunks (collective-free
        # PE work that covers the AllReduce round-trip) ----
        qsbs, gsbs = [], []
        for ch in range(NCH):
            csl = ts(ch, CHUNK)
            qsb = pb_qg.tile([128, 8, CHUNK], DT, tag=f"qsb{ch}")
            gsb = pb_qg.tile([128, 8, CHUNK], DT, tag=f"gsb{ch}")
            qsbs.append(qsb)
            gsbs.append(gsb)
            for p in range(8):
                proj_block(p, csl, qsb, gsb, "q")
            if ch == 0:
                nc.sync.dma_start(out=kvf[:], in_=cc_out.ap()[:, :, :])
            for p in range(8):
                proj_block(p, csl, qsb, gsb, "g")

        ps_proj_cm.__exit__(None, None, None)

        # ---- collective prep + scheduling gate, ALL on the Pool engine.
        # No pass-1 matmul ever waits on Pool completions, so these
        # collective-dependent ops cannot poison pass-1 counting-semaphore
        # thresholds (they did when placed on ACT or DVE). ksd2/kvb2 gain a
        # zero-valued dependency on the LAST projection block's output:
        # the Tile scheduler's readiness model ignores AllReduce latency
        # and would otherwise hoist the first collective-dependent matmul
        # right behind chunk 0's projections, head-blocking the in-order
        # PE queue on the collective.
        # Every prep op that reads kvf is ALSO data-gated (via a zero add
        # operand) on chunk 2's projections: an ungated op here gets
        # emitted mid-Pool-queue by the scheduler's optimistic model and
        # head-blocks Pool on the AllReduce, which stalls pass-1 DVE relu
        # ops (they WAR-wait Pool's qsb-add drains) and thus the PE.
        # Gating on chunk 2 (75% of pass 1) keeps ~80us of collective
        # coverage while leaving the final chunk to hide this prep.
        # ksd carries plain k_sum (SCALE folded into the z chain) so the
        # scatter can be 2-operand gated adds.
        prep_prio = tc.high_priority(offset=-1000000)
        prep_prio.__enter__()
        zlg = pb_small.tile([128, 65], F32, tag="zlg")
        nc.gpsimd.tensor_scalar_mul(zlg, gsbs[2][:, 7, 0:65], 0.0)
        nc.gpsimd.memset(ksd[:], 0.0)
        for p in range(8):
            nc.gpsimd.tensor_add(kvb[:, p, :], kvf[:, p, :], zlg)
            nc.gpsimd.tensor_add(ksd[0:64, p, 2 * p:2 * p + 1],
                                 kvf[0:64, p, 64:65], zlg[0:64, 0:1])
            nc.gpsimd.tensor_add(ksd[64:128, p, 2 * p + 1:2 * p + 2],
                                 kvf[64:128, p, 64:65], zlg[64:128, 0:1])
        ksd2 = persist.tile([128, 8, 16], DT, tag="ksd2")
        kvb2 = persist.tile([128, 8, 65], DT, tag="kvb2")
        zl65 = pb_small.tile([128, 65], DT, tag="zl65")
        # gate on p6, not p7: frees the gate chain to overlap the final
        # projection block (still ~107us of guaranteed collective coverage)
        nc.gpsimd.tensor_scalar_mul(zl65, gsbs[NCH - 1][:, 6, 0:65], 0.0)
        for p in range(8):
            nc.gpsimd.tensor_add(ksd2[:, p, :], ksd[:, p, :],
                                 zl65[:, 0:16])
            nc.gpsimd.tensor_add(kvb2[:, p, :], kvb[:, p, :], zl65)
        prep_prio.__exit__(None, None, None)

        ps_ops_cm = tc.tile_pool(name="ps_ops", bufs=2, space="PSUM")
        ps_z_cm = tc.tile_pool(name="ps_z", bufs=2, space="PSUM")
        ps_qk_cm = tc.tile_pool(name="ps_qk", bufs=2, space="PSUM")
        ps_y_cm = tc.tile_pool(name="ps_y", bufs=2, space="PSUM")
        ps_ops = ps_ops_cm.__enter__()
        ps_z = ps_z_cm.__enter__()
        ps_qk = ps_qk_cm.__enter__()
        ps_y = ps_y_cm.__enter__()

        # ---- pass 2: attention + output projection per chunk; each
        # chunk's qk runs one chunk ahead so the z reciprocal chain (DVE)
        # hides under the previous chunk's y matmuls. The whole pass is
        # deprioritized far past pass 1 so the scheduler's ready-heap
        # always prefers projection matmuls over collective-dependent ones.
        prio_cm = tc.high_priority(offset=-1000000)
        prio_cm.__enter__()

        def qk_mms(ch):
            qkps = ps_qk.tile([16, CHUNK], F32, tag="qk")
            for p in range(8):
                nc.tensor.matmul(
                    qkps, mm(ksd2[:, p, :]), mm(qsbs[ch][:, p, :]),
                    start=(p == 0), stop=(p == 7),
                    skip_group_check=True,
                )
            return qkps

        qkps_next = qk_mms(0)
        for ch in range(NCH):
            csl = ts(ch, CHUNK)
            qsb, gsb = qsbs[ch], gsbs[ch]
            qkps = qkps_next
            zq = pb_tmp.tile([16, CHUNK], F32, tag="zq")
            nc.vector.tensor_scalar_max(zq, qkps, CLAMP)
            zr = pb_tmp.tile([16, CHUNK], F32, tag="zr")
            nc.vector.reciprocal(zr, zq)
            zqr = pb_tmp.tile([16, CHUNK], DT, tag="zqr")
            nc.vector.tensor_scalar_mul(zqr, zr, SCALE)

            asb = pb_big.tile([128, 8, CHUNK], DT, tag="asb")
            for p in range(8):
                ops_ = ps_ops.tile([128, CHUNK], F32, tag="ops")
                for rr in range(2):
                    pr = slice(64 * rr, 64 * rr + 64)
                    nc.tensor.matmul(
                        ops_[pr, :], mm(kvb2[pr, p, 0:64]),
                        mm(qsb[pr, p, :]),
                        start=True, stop=True,
                    )
                zbps = ps_z.tile([128, CHUNK], F32, tag="z")
                nc.tensor.matmul(zbps, mm(sel[:, p, :]), mm(zqr),
                                 start=True, stop=True)
                t1 = pb_tmp.tile([128, CHUNK], F32, tag="bt1")
                # each mul reads at most one PSUM operand (HW restriction)
                nc.vector.tensor_mul(t1, ops_, gsb[:, p, :])
                nc.vector.tensor_mul(asb[:, p, :], t1, zbps)
                if p == 1 and ch + 1 < NCH:
                    qkps_next = qk_mms(ch + 1)

            for d in range(8):
                yps = ps_y.tile([128, CHUNK], F32, tag="y")
                for fi in range(8):
                    nc.tensor.matmul(
                        yps, mm(wsb["wo"][:, fi, ts(d, 128)]),
                        mm(asb[:, fi, :]),
                        start=(fi == 0), stop=(fi == 7),
                    )
                ysb = pb_tmp.tile([128, CHUNK], DT, tag="ysb")
                nc.scalar.copy(ysb, yps)
                nc.sync.dma_start(out=y_d.ap()[ts(d, 128), csl],
                                  in_=ysb[:])

        prio_cm.__exit__(None, None, None)
        ps_y_cm.__exit__(None, None, None)
        ps_qk_cm.__exit__(None, None, None)
        ps_z_cm.__exit__(None, None, None)
        ps_ops_cm.__exit__(None, None, None)


def _np_dt(dt_mode):
    return ml_dtypes.bfloat16 if dt_mode == "bf16" else np.float32


def prep_inputs(x, Wq, Wk, Wv, Wg, bg, Wo, dt_mode=DT_MODE):
    npdt = _np_dt(dt_mode)
    f8 = ml_dtypes.float8_e4m3
    x_f = np.ascontiguousarray(np.asarray(x, np.float32).reshape(B * N, DIM))
    w_t = {}
    w_t["wv"] = np.ascontiguousarray(
        np.asarray(Wv, np.float32).T).astype(npdt)
    for nm, W in (("wq8", Wq), ("wk8", Wk), ("wg8", Wg)):
        w_t[nm] = np.ascontiguousarray(
            np.asarray(W, np.float32).T).astype(f8)
    w_t["wo"] = np.ascontiguousarray(
        np.asarray(Wo, np.float32).T).astype(npdt)
    bg_f = np.ascontiguousarray(np.asarray(bg, np.float32))
    in_maps = []
    for c in range(N_CORES):
        xt_T = np.ascontiguousarray(x_f[c * TPC:(c + 1) * TPC].T)
        m = {"xt": xt_T.astype(npdt), "x8": xt_T.astype(f8), "bg": bg_f}
        m.update(w_t)
        in_maps.append(m)
    return in_maps


def unshard_output(y_parts):
    out = np.empty((B * N, DIM), np.float32)
    for c in range(N_CORES):
        out[c * TPC:(c + 1) * TPC] = np.asarray(y_parts[c], np.float32).T
    return out.reshape(B, N, DIM)


def get_nc(dt_mode=DT_MODE):
    key = ("nc", dt_mode)
    if key not in _CACHE:
        _CACHE[key] = _build(dt_mode)
    return _CACHE[key]


def kernel(x, Wq, Wk, Wv, Wg, bg, Wo):
    from concourse.bass_utils import run_bass_kernel_spmd

    nc = get_nc()
    in_maps = prep_inputs(x, Wq, Wk, Wv, Wg, bg, Wo)
    res = run_bass_kernel_spmd(nc, in_maps, core_ids=list(range(N_CORES)))
    return unshard_output([res.results[c]["y"] for c in range(N_CORES)])


# revision 63
# speedup vs baseline: 1.2653x; 1.0033x over previous
"""Trainium2 Bass kernel for nn_GatedAttention (linear attention with sigmoid
gate).

Strategy: shard the 16384 token rows across 8 cores (2048 each; cores 2b,2b+1
hold batch b). Per core, two phases:
  A: K,V projections (token-major) + per-head kv' = K^T [V|1] accumulated in
     persistent PSUM tiles over all local tokens (the ones column folds k_sum
     into kv'). kv matmuls run one m-tile behind the projections so the elu
     chain never stalls the PE.
  -- pairwise AllReduce of kv' between the two cores sharing a batch --
  B: Q,G projections (feature-major), out^T = kv'^T @ Q per head, normalizer
     z = SCALE/max(q.k_sum,eps) applied via tiny selector matmuls, gate, and
     the final output projection, all feature-major.
DMAs are issued in consumer order (X/wk/wv before phase A, wq/wg/wo/bg/sel
after the first m-tile, collective output after ch0's Q matmuls) so counting
semaphores never serialize the PE behind unrelated transfers.
Host transposes x to feature-major and pre-transposes weights; output returns
feature-major bf16 per-core slabs that the host transposes back.
"""
import sys

sys.path.insert(0, "/opt/trn_rl_repo")

import numpy as np
import ml_dtypes

B, N, DIM = 4, 4096, 1024
HEADS, DH = 16, 64
SCALE = DH ** -0.5
N_CORES = 8
TPC = B * N // N_CORES      # 2048 tokens per core
NMT = TPC // 128            # 16 m-tiles (phase A)
CHUNK = 512
NCH = TPC // CHUNK          # 4 chunks (phase B)
CLAMP = 1e-6  # ksd carries plain k_sum; SCALE applied in the zqr multiply

DT_MODE = "bf16"            # "bf16" | "f32r" | "f32"

_CACHE = {}


def _build(dt_mode=DT_MODE, reps=1):
    import concourse.bacc as bacc
    import concourse.bass as bass
    import concourse.tile as tile
    from concourse import mybir

    AF = mybir.ActivationFunctionType
    F32 = mybir.dt.float32
    F8 = mybir.dt.float8e4
    DR = mybir.MatmulPerfMode.DoubleRow
    DT = mybir.dt.bfloat16 if dt_mode == "bf16" else mybir.dt.float32

    def mm(ap):
        return ap.bitcast(mybir.dt.float32r) if dt_mode == "f32r" else ap

    ts = bass.ts

    nc = bacc.Bacc("TRN2", target_bir_lowering=False, debug=False,
                   num_devices=N_CORES)
    xt = nc.dram_tensor("xt", [DIM, TPC], DT, kind="ExternalInput")
    x8_d = nc.dram_tensor("x8", [DIM, TPC], F8, kind="ExternalInput")
    w_in = {}
    for nm in ("wv", "wo"):
        w_in[nm] = nc.dram_tensor(nm, [DIM, DIM], DT, kind="ExternalInput")
    for nm in ("wk8", "wq8", "wg8"):
        w_in[nm] = nc.dram_tensor(nm, [DIM, DIM], F8, kind="ExternalInput")
    bg_d = nc.dram_tensor("bg", [DIM], F32, kind="ExternalInput")
    y_d = nc.dram_tensor("y", [DIM, TPC], DT, kind="ExternalOutput")
    cc_in = nc.dram_tensor("cc_in", [128, 8, 65], F32)
    cc_out = nc.dram_tensor("cc_out", [128, 8, 65], F32)

    with tile.TileContext(nc, num_cores=N_CORES) as tc:
        with (
            tc.tile_pool(name="persist", bufs=1) as persist,
            tc.tile_pool(name="pb_big", bufs=2) as pb_big,
        ):
            X = persist.tile([128, 8, TPC], DT, tag="x")
            X8 = persist.tile([128, 8, TPC], F8, tag="x8")
            wsb = {}
            wsb["wo"] = persist.tile([128, 8, DIM], DT, tag="wo", name="wo")
            for nm in ("wq8", "wg8"):
                wsb[nm] = persist.tile([128, 8, DIM], F8, tag=nm, name=nm)
            bg_sb = persist.tile([128, 8], F32, tag="bg")
            sel_np = np.zeros((16, 8, 128), _np_dt(dt_mode))
            for p in range(8):
                sel_np[2 * p, p, 0:64] = 1.0
                sel_np[2 * p + 1, p, 64:128] = 1.0
            sel_d = nc.inline_tensor(sel_np, name="sel_const")
            sel = persist.tile([16, 8, 128], DT, tag="sel")

            for _rep in range(reps):
                _phases(nc, tc, bass, mybir, AF, F32, DT, mm, ts, X, wsb,
                        bg_sb, sel, sel_d, w_in, xt, bg_d, cc_in, cc_out, y_d,
                        tc_pools=(persist, pb_big), X8=X8, x8_d=x8_d,
                        F8=F8, DR=DR)
    nc.compile()
    return nc


def _phases(nc, tc, bass, mybir, AF, F32, DT, mm, ts, X, wsb, bg_sb, sel,
            sel_d, w_in, xt, bg_d, cc_in, cc_out, y_d, tc_pools, X8, x8_d,
            F8, DR):
    persist, pb_big = tc_pools
    # ---------------- phase A ----------------
    with (
        tc.tile_pool(name="pa_w", bufs=1) as pa_w,
        tc.tile_pool(name="pa_tmp", bufs=2) as pa_tmp,
        tc.tile_pool(name="pa_ps", bufs=2, space="PSUM") as pa_ps,
        tc.tile_pool(name="kv_ps", bufs=1, space="PSUM") as kv_pool,
    ):
        wsb["wk8"] = pa_w.tile([128, 8, DIM], F8, tag="wk8", name="wk8")
        wsb["wv"] = pa_w.tile([128, 8, DIM], DT, tag="wv", name="wv")
        # consumer-ordered DMA issue: phase A inputs first, interleaved by
        # contraction chunk so the first m-tile can start ASAP (X/wv are
        # only needed once the first V-projection starts, ~1.7us after K).
        for i in range(8):
            nc.sync.dma_start(out=X8[:, i, :], in_=x8_d.ap()[ts(i, 128), :])
            nc.sync.dma_start(out=wsb["wk8"][:, i, :],
                              in_=w_in["wk8"].ap()[ts(i, 128), :])
        for i in range(8):
            nc.sync.dma_start(out=X[:, i, :], in_=xt.ap()[ts(i, 128), :])
            nc.sync.dma_start(out=wsb["wv"][:, i, :],
                              in_=w_in["wv"].ap()[ts(i, 128), :])

        # HAM warm-up: the PE clock sits at 1.2GHz until ~3.4us of sustained
        # matmul activity, and the input DMAs take ~12us to land AND pace
        # the first two m-tiles. Dummy matmuls on zeroed scratch fill the
        # initial window, and more are interleaved into the DMA-paced
        # m-tiles (below) so the activity window never sees an idle gap.
        warm_cm = tc.tile_pool(name="warm", bufs=1)
        warm_pool = warm_cm.__enter__()
        warm_ps_cm = tc.tile_pool(name="warm_ps", bufs=1, space="PSUM")
        warm_ps = warm_ps_cm.__enter__()
        wa = warm_pool.tile([128, 128], DT, tag="wa")
        wb = warm_pool.tile([128, 512], DT, tag="wb")
        nc.vector.memset(wa[:], 0.0)
        nc.vector.memset(wb[:], 0.0)
        wp = warm_ps.tile([128, 512], F32, tag="wp")
        for _ in range(14):
            nc.tensor.matmul(wp, mm(wa), mm(wb), start=True, stop=True)

        # persistent PSUM accumulators for kv': 2 tiles x [128, 4, 128]
        # (tile w holds heads 8w..8w+7: slice j rows 0:64 = head 8w+2j,
        #  rows 64:128 = head 8w+2j+1; j-slices padded to 128 floats so a
        #  matmul output never crosses a PSUM bank boundary)
        kv_ps = [kv_pool.tile([128, 4, 128], F32, tag=f"kv{w}",
                              name=f"kv{w}") for w in range(2)]
        ksb_hist = [None, None]
        vp_hist = [None, None]

        def kv_mms(mt):
            ksb_o = ksb_hist[mt % 2]
            vp_o = vp_hist[mt % 2]
            for w in range(2):
                for j in range(4):
                    for c in range(2):
                        h = 8 * w + 2 * j + c
                        # start only on the FIRST matmul touching this bank's
                        # partition plane: start_tensor_calc marks the whole
                        # 2KB zero-region pending, so a second start=True in
                        # the same bank would re-poison already-written
                        # slices and turn later accumulates into overwrites.
                        nc.tensor.matmul(
                            kv_ps[w][64 * c:64 * c + 64, j, 0:65],
                            mm(ksb_o[:, ts(h, 64)]),
                            mm(vp_o[:, h, :]),
                            start=(mt == 0 and j == 0),
                            stop=(mt == NMT - 1),
                            skip_group_check=True,
                        )

        for mt in range(NMT):
            msl = ts(mt, 128)
            kps = pa_ps.tile([128, 1024], F32, tag="proj")
            # K projection in fp8 DoubleRow: contraction pairs of 128-chunks
            # (effective K=256 per matmul, ~1.4-2x PE throughput); fp8 on
            # Q/K is accuracy-safe because the normalizer z cancels most of
            # the quantization error (measured rel 0.0069 vs 0.02 budget).
            for j in range(4):
                for o in range(2):
                    nc.tensor.matmul(
                        kps[:, ts(o, 512)],
                        X8[:, 2 * j:2 * j + 2, msl],
                        wsb["wk8"][:, 2 * j:2 * j + 2, ts(o, 512)],
                        start=(j == 0), stop=(j == 3),
                        perf_mode=DR,
                    )
                if mt < 2:
                    nc.tensor.matmul(wp, mm(wa), mm(wb), start=True,
                                     stop=True)
            if mt == 1:
                # phase-B inputs: issued after the first m-tile's matmuls so
                # phase-A waits never count these transfers.
                bg_ap = bg_d.ap()
                nc.sync.dma_start(
                    out=bg_sb[:],
                    in_=bass.AP(tensor=bg_ap.tensor, offset=0,
                                ap=[[1, 128], [128, 8]]),
                )
                nc.sync.dma_start(out=sel[:], in_=sel_d.ap())
                for i in range(8):
                    for nm in ("wq8", "wg8", "wo"):
                        nc.sync.dma_start(out=wsb[nm][:, i, :],
                                          in_=w_in[nm].ap()[ts(i, 128), :])
            r1 = pa_tmp.tile([128, 1024], F32, tag="r1")
            nc.scalar.activation(r1, kps, AF.Relu)
            m1 = pa_tmp.tile([128, 1024], F32, tag="m1")
            nc.vector.tensor_scalar_min(m1, kps, 0.0)
            e1 = pa_tmp.tile([128, 1024], F32, tag="e1")
            nc.scalar.activation(e1, m1, AF.Exp)
            ksb = pa_tmp.tile([128, 1024], DT, tag="ksb")
            nc.gpsimd.tensor_add(ksb, r1, e1)
            ksb_hist[mt % 2] = ksb

            vps = pa_ps.tile([128, 16, 64], F32, tag="proj")
            for i in range(8):
                for o in range(2):
                    nc.tensor.matmul(
                        vps[:, ts(o, 8), :],
                        mm(X[:, i, msl]),
                        mm(wsb["wv"][:, i, ts(o, 512)]),
                        start=(i == 0), stop=(i == 7),
                    )
                if mt < 2:
                    nc.tensor.matmul(wp, mm(wa), mm(wb), start=True,
                                     stop=True)
            vp = pa_tmp.tile([128, 16, 65], DT, tag="vp")
            nc.vector.memset(vp[:, :, 64:65], 1.0)
            nc.scalar.copy(vp[:, :, 0:64], vps[:, :, :])
            vp_hist[mt % 2] = vp

            if mt > 0:
                kv_mms(mt - 1)
        kv_mms(NMT - 1)
        warm_ps_cm.__exit__(None, None, None)
        warm_cm.__exit__(None, None, None)

        kv_sb = pa_tmp.tile([128, 8, 65], F32, tag="kv_sb", bufs=1,
                            name="kv_sb")
        for w in range(2):
            nc.vector.tensor_copy(kv_sb[:, 4 * w:4 * w + 4, :],
                                  kv_ps[w][:, :, 0:65])
        nc.sync.dma_start(out=cc_in.ap()[:, :, :], in_=kv_sb[:])

    nc.gpsimd.collective_compute(
        "AllReduce",
        mybir.AluOpType.add,
        replica_groups=[[0, 1], [2, 3], [4, 5], [6, 7]],
        ins=[cc_in.ap().opt()],
        outs=[cc_out.ap().opt()],
    )

    # ---------------- phase B ----------------
    with (
        tc.tile_pool(name="pb_tmp", bufs=2) as pb_tmp,
        tc.tile_pool(name="pb_small", bufs=1) as pb_small,
        tc.tile_pool(name="pb_qg", bufs=1) as pb_qg,
    ):
        # collective results live in the persist pool: fresh SBUF, so the
        # kvf DMA has no write-after-read wait on phase-A consumers.
        kvf = persist.tile([128, 8, 65], F32, tag="kvf")
        kvb = persist.tile([128, 8, 65], DT, tag="kvb")
        ksd = persist.tile([128, 8, 16], DT, tag="ksd")

        ps_proj_cm = tc.tile_pool(name="ps_proj", bufs=6, space="PSUM")
        ps_proj = ps_proj_cm.__enter__()

        def proj_block(p, csl, qsb, gsb, which):
            pps = ps_proj.tile([128, CHUNK], F32, tag="proj")
            wname = "wq8" if which == "q" else "wg8"
            # fp8 DoubleRow (see K projection note; G measured rel 0.0127
            # alone, 0.0162 combined with Q/K — inside the 0.02 budget)
            for j in range(4):
                nc.tensor.matmul(
                    pps, wsb[wname][:, 2 * j:2 * j + 2, ts(p, 128)],
                    X8[:, 2 * j:2 * j + 2, csl],
                    start=(j == 0), stop=(j == 3),
                    perf_mode=DR,
                )
            if which == "q":
                # relu on DVE, not ACT: with fp8 projections the PE pace is
                # ~0.85us/block and an ACT-side relu+exp chain (1.2us) makes
                # pass 1 ACT-bound (PSUM-reuse stalls + HAM oscillation).
                r1 = pb_tmp.tile([128, CHUNK], F32, tag="br1")
                nc.vector.tensor_scalar_max(r1, pps, 0.0)
                m1 = pb_tmp.tile([128, CHUNK], F32, tag="bm1")
                nc.vector.tensor_scalar_min(m1, pps, 0.0)
                e1 = pb_tmp.tile([128, CHUNK], F32, tag="be1")
                nc.scalar.activation(e1, m1, AF.Exp)
                nc.gpsimd.tensor_add(qsb[:, p, :], r1, e1)
            else:
                nc.scalar.activation(gsb[:, p, :], pps, AF.Sigmoid,
                                     bias=bg_sb[:, p:p + 1])

        # ---- pass 1: Q and G projections for ALL chunks (collective-free
        # PE work that covers the AllReduce round-trip) ----
        qsbs, gsbs = [], []
        for ch in range(NCH):
            csl = ts(ch, CHUNK)
            qsb = pb_qg.tile([128, 8, CHUNK], DT, tag=f"qsb{ch}")
            gsb = pb_qg.tile([128, 8, CHUNK], DT, tag=f"gsb{ch}")
            qsbs.append(qsb)
            gsbs.append(gsb)
            # Q and G blocks interleaved: a straight 8-block Q run outpaces
            # DVE (2 relu/min ops per Q block, none per G block) and stalls
            # the PE on PSUM-reuse every chunk; per Q+G pair the PE takes
            # 1.7us vs DVE's 1.38us, so the pair loop never stalls.
            for p in range(8):
                proj_block(p, csl, qsb, gsb, "q")
                proj_block(p, csl, qsb, gsb, "g")
                if ch == 0 and p == 0:
                    nc.sync.dma_start(out=kvf[:],
                                      in_=cc_out.ap()[:, :, :])

        ps_proj_cm.__exit__(None, None, None)

        # ---- collective prep + scheduling gate, ALL on the Pool engine.
        # No pass-1 matmul ever waits on Pool completions, so these
        # collective-dependent ops cannot poison pass-1 counting-semaphore
        # thresholds (they did when placed on ACT or DVE). ksd2/kvb2 gain a
        # zero-valued dependency on the LAST projection block's output:
        # the Tile scheduler's readiness model ignores AllReduce latency
        # and would otherwise hoist the first collective-dependent matmul
        # right behind chunk 0's projections, head-blocking the in-order
        # PE queue on the collective.
        # Every prep op that reads kvf is ALSO data-gated (via a zero add
        # operand) on chunk 2's projections: an ungated op here gets
        # emitted mid-Pool-queue by the scheduler's optimistic model and
        # head-blocks Pool on the AllReduce, which stalls pass-1 DVE relu
        # ops (they WAR-wait Pool's qsb-add drains) and thus the PE.
        # Gating on chunk 2 (75% of pass 1) keeps ~80us of collective
        # coverage while leaving the final chunk to hide this prep.
        # ksd carries plain k_sum (SCALE folded into the z chain) so the
        # scatter can be 2-operand gated adds.
        prep_prio = tc.high_priority(offset=-1000000)
        prep_prio.__enter__()
        zlg = pb_small.tile([128, 65], F32, tag="zlg")
        nc.gpsimd.tensor_scalar_mul(zlg, gsbs[2][:, 7, 0:65], 0.0)
        nc.gpsimd.memset(ksd[:], 0.0)
        for p in range(8):
            nc.gpsimd.tensor_add(kvb[:, p, :], kvf[:, p, :], zlg)
            nc.gpsimd.tensor_add(ksd[0:64, p, 2 * p:2 * p + 1],
                                 kvf[0:64, p, 64:65], zlg[0:64, 0:1])
            nc.gpsimd.tensor_add(ksd[64:128, p, 2 * p + 1:2 * p + 2],
                                 kvf[64:128, p, 64:65], zlg[64:128, 0:1])
        ksd2 = persist.tile([128, 8, 16], DT, tag="ksd2")
        kvb2 = persist.tile([128, 8, 65], DT, tag="kvb2")
        zl65 = pb_small.tile([128, 65], DT, tag="zl65")
        # gate on p4, not p7: the gate chain overlaps the last three
        # projection pairs (~5us) instead of serializing after them; the
        # guaranteed collective coverage is still ~50us of projection work.
        nc.gpsimd.tensor_scalar_mul(zl65, gsbs[NCH - 1][:, 4, 0:65], 0.0)
        for p in range(8):
            nc.gpsimd.tensor_add(ksd2[:, p, :], ksd[:, p, :],
                                 zl65[:, 0:16])
            nc.gpsimd.tensor_add(kvb2[:, p, :], kvb[:, p, :], zl65)
        prep_prio.__exit__(None, None, None)

        ps_ops_cm = tc.tile_pool(name="ps_ops", bufs=2, space="PSUM")
        ps_z_cm = tc.tile_pool(name="ps_z", bufs=2, space="PSUM")
        ps_qk_cm = tc.tile_pool(name="ps_qk", bufs=2, space="PSUM")
        ps_y_cm = tc.tile_pool(name="ps_y", bufs=2, space="PSUM")
        ps_ops = ps_ops_cm.__enter__()
        ps_z = ps_z_cm.__enter__()
        ps_qk = ps_qk_cm.__enter__()
        ps_y = ps_y_cm.__enter__()

        # ---- pass 2: attention + output projection per chunk; each
        # chunk's qk runs one chunk ahead so the z reciprocal chain (DVE)
        # hides under the previous chunk's y matmuls. The whole pass is
        # deprioritized far past pass 1 so the scheduler's ready-heap
        # always prefers projection matmuls over collective-dependent ones.
        prio_cm = tc.high_priority(offset=-1000000)
        prio_cm.__enter__()

        def qk_mms(ch):
            qkps = ps_qk.tile([16, CHUNK], F32, tag="qk")
            for p in range(8):
                nc.tensor.matmul(
                    qkps, mm(ksd2[:, p, :]), mm(qsbs[ch][:, p, :]),
                    start=(p == 0), stop=(p == 7),
                    skip_group_check=True,
                )
            return qkps

        qkps_next = qk_mms(0)
        for ch in range(NCH):
            csl = ts(ch, CHUNK)
            qsb, gsb = qsbs[ch], gsbs[ch]
            qkps = qkps_next
            zq = pb_tmp.tile([16, CHUNK], F32, tag="zq")
            nc.vector.tensor_scalar_max(zq, qkps, CLAMP)
            zr = pb_tmp.tile([16, CHUNK], F32, tag="zr")
            nc.vector.reciprocal(zr, zq)
            zqr = pb_tmp.tile([16, CHUNK], DT, tag="zqr")
            nc.vector.tensor_scalar_mul(zqr, zr, SCALE)

            asb = pb_big.tile([128, 8, CHUNK], DT, tag="asb")
            for p in range(8):
                ops_ = ps_ops.tile([128, CHUNK], F32, tag="ops")
                for rr in range(2):
                    pr = slice(64 * rr, 64 * rr + 64)
                    nc.tensor.matmul(
                        ops_[pr, :], mm(kvb2[pr, p, 0:64]),
                        mm(qsb[pr, p, :]),
                        start=True, stop=True,
                    )
                zbps = ps_z.tile([128, CHUNK], F32, tag="z")
                nc.tensor.matmul(zbps, mm(sel[:, p, :]), mm(zqr),
                                 start=True, stop=True)
                t1 = pb_tmp.tile([128, CHUNK], F32, tag="bt1")
                # each mul reads at most one PSUM operand (HW restriction)
                nc.vector.tensor_mul(t1, ops_, gsb[:, p, :])
                nc.vector.tensor_mul(asb[:, p, :], t1, zbps)
                if p == 1 and ch + 1 < NCH:
                    qkps_next = qk_mms(ch + 1)

            for d in range(8):
                yps = ps_y.tile([128, CHUNK], F32, tag="y")
                for fi in range(8):
                    nc.tensor.matmul(
                        yps, mm(wsb["wo"][:, fi, ts(d, 128)]),
                        mm(asb[:, fi, :]),
                        start=(fi == 0), stop=(fi == 7),
                    )
                ysb = pb_tmp.tile([128, CHUNK], DT, tag="ysb")
                nc.scalar.copy(ysb, yps)
                nc.sync.dma_start(out=y_d.ap()[ts(d, 128), csl],
                                  in_=ysb[:])

        prio_cm.__exit__(None, None, None)
        ps_y_cm.__exit__(None, None, None)
        ps_qk_cm.__exit__(None, None, None)
        ps_z_cm.__exit__(None, None, None)
        ps_ops_cm.__exit__(None, None, None)


def _np_dt(dt_mode):
    return ml_dtypes.bfloat16 if dt_mode == "bf16" else np.float32


def prep_inputs(x, Wq, Wk, Wv, Wg, bg, Wo, dt_mode=DT_MODE):
    npdt = _np_dt(dt_mode)
    f8 = ml_dtypes.float8_e4m3
    x_f = np.ascontiguousarray(np.asarray(x, np.float32).reshape(B * N, DIM))
    w_t = {}
    w_t["wv"] = np.ascontiguousarray(
        np.asarray(Wv, np.float32).T).astype(npdt)
    for nm, W in (("wq8", Wq), ("wk8", Wk), ("wg8", Wg)):
        w_t[nm] = np.ascontiguousarray(
            np.asarray(W, np.float32).T).astype(f8)
    w_t["wo"] = np.ascontiguousarray(
        np.asarray(Wo, np.float32).T).astype(npdt)
    bg_f = np.ascontiguousarray(np.asarray(bg, np.float32))
    in_maps = []
    for c in range(N_CORES):
        xt_T = np.ascontiguousarray(x_f[c * TPC:(c + 1) * TPC].T)
        m = {"xt": xt_T.astype(npdt), "x8": xt_T.astype(f8), "bg": bg_f}
        m.update(w_t)
        in_maps.append(m)
    return in_maps


def unshard_output(y_parts):
    out = np.empty((B * N, DIM), np.float32)
    for c in range(N_CORES):
        out[c * TPC:(c + 1) * TPC] = np.asarray(y_parts[c], np.float32).T
    return out.reshape(B, N, DIM)


def get_nc(dt_mode=DT_MODE):
    key = ("nc", dt_mode)
    if key not in _CACHE:
        _CACHE[key] = _build(dt_mode)
    return _CACHE[key]


def kernel(x, Wq, Wk, Wv, Wg, bg, Wo):
    from concourse.bass_utils import run_bass_kernel_spmd

    nc = get_nc()
    in_maps = prep_inputs(x, Wq, Wk, Wv, Wg, bg, Wo)
    res = run_bass_kernel_spmd(nc, in_maps, core_ids=list(range(N_CORES)))
    return unshard_output([res.results[c]["y"] for c in range(N_CORES)])
